# revision 3
# baseline (speedup 1.0000x reference)
"""Trainium2 Bass kernel for nn_AttnBlock (sparse 3x3-window attention block).

Structure (per core, one batch image):
  - LN1/qkv computed once per padded pixel (34x34 grid); g1/beta1 folded into
    w_qkv/b_qkv and g2/beta2 into w_fc/b_fc on the host, so LN applies are two
    plain DVE ops.
  - Scores deduplicated into 25 displacement maps E_e = q . shift_e(k)
    (DVE/Pool elementwise product + PE per-head column reduction).
  - Softmax denominators via box-sum selection matmul (gsel, scaled x9 to fold
    the window mean), W = sum_ki (wsel_ki @ F)(shift) * R-broadcast.
  - o_mean = sum_kj shift(v * W-broadcast); broadcasts via PE selection
    matmuls, PSUM->SBUF bf16 copies on Act/Pool so big DVE products run in
    2x (16-bit) mode, full width.
  - Box filter (residual mean) runs on the Pool engine under the qkv phase.

Sharding: data-parallel over batch B=8 -> one batch per NeuronCore.
"""

import functools
import numpy as np
import ml_dtypes

import concourse.bass as bass
import concourse.mybir as mybir
import concourse.tile as tile
from concourse import bacc
from concourse.bass_utils import run_bass_kernel_spmd

F32 = mybir.dt.float32
BF16 = mybir.dt.bfloat16
AF = mybir.ActivationFunctionType
ALU = mybir.AluOpType

C = 768
NCH = 6          # channel chunks of 128
G = 34           # padded grid side
A = G * G        # 1156 padded pixels
AW = 1160        # padded-pixel width with 4 pad cols
NW = 1088        # window-grid width = 32*34 (rows 0..31, cols 0..33)
KW = 1300        # k map width with +-70 margins (content at 70)
HEADS = 8
HD = 96
SCALE = HD ** -0.5
EPS = 1e-5

SEG_A = [(0, 386), (386, 386), (772, 384)]   # PSUM segs over the a-grid
SEG_N = [(0, 363), (363, 363), (726, 362)]   # PSUM segs over the n-grid

E_LIST = [(er, ec) for er in range(-2, 3) for ec in range(-2, 3)]  # 25
KI_LIST = [(r, c) for r in range(3) for c in range(3)]             # 9

POOL_EI = {4, 9, 14, 19, 24}  # displacement products offloaded to Pool


def emit_kernel(ctx, tc, ins, outs):
    nc = tc.nc
    xp_d = ins["xp"]          # [6,128,1156] bf16
    wq_d = ins["wqkv"]        # [6,128,2304] bf16 (g1-folded)
    wp_d = ins["wproj"]       # [6,128,768] bf16
    wf_d = ins["wfc"]         # [6,128,256] bf16 (g2-folded)
    bqkv_d = ins["bqkv"]      # [128,18] f32 (beta1-folded)
    bproj_d = ins["bproj"]    # [128,6] f32
    bfc_d = ins["bfc"]        # [128,2] f32 (beta2-folded)
    selqk_d = ins["selqk"]    # [6,128,248] bf16
    gsel0_d = ins["gsel0"]    # [128,72] bf16 (x9)
    gsel1_d = ins["gsel1"]    # [72,72] bf16 (x9)
    wsel0_d = ins["wsel0"]    # [128,9,72] bf16
    wsel1_d = ins["wsel1"]    # [72,9,72] bf16
    repsel9_d = ins["repsel9"]  # [72,72] bf16
    csel_d = ins["csel"]      # [72,128] bf16
    ident_d = ins["ident"]    # [128,128] bf16
    onesk_d = ins["onesk"]    # [128,1] bf16
    y_d = outs["y"]           # [2,128,32,32] f32

    consts = ctx.enter_context(tc.tile_pool(name="consts", bufs=1))
    big = ctx.enter_context(tc.tile_pool(name="big", bufs=1))
    prodp = ctx.enter_context(tc.tile_pool(name="prodp", bufs=5))
    brp = ctx.enter_context(tc.tile_pool(name="brp", bufs=2))
    small = ctx.enter_context(tc.tile_pool(name="small", bufs=1))
    psA = ctx.enter_context(tc.tile_pool(name="psA", bufs=1, space="PSUM"))
    psB = ctx.enter_context(tc.tile_pool(name="psB", bufs=1, space="PSUM"))
    psC = ctx.enter_context(tc.tile_pool(name="psC", bufs=2, space="PSUM"))
    drp = ctx.enter_context(tc.tile_pool(name="drp", bufs=2, space="DRAM"))

    def psa():
        return psA.tile([128, 3, 512], F32, tag="a", name="psa_t")

    def psb():
        return psB.tile([128, 3, 512], F32, tag="b", name="psb_t")

    # two-op PSUM->SBUF copy helpers (banks 0-1 fused, bank 2), SEG-uniform
    def copy2(eng, dst01, dst2, ps, segs, m=128, func=AF.Copy, bias=None,
              scale=1.0):
        w01, w2 = segs[0][1], segs[2][1]
        if eng == "act":
            kw = {} if bias is None else {"bias": bias}
            nc.scalar.activation(dst01[0:m], ps[0:m, 0:2, 0:w01], func,
                                 scale=scale, **kw)
            nc.scalar.activation(dst2[0:m], ps[0:m, 2, 0:w2], func,
                                 scale=scale, **kw)
        elif eng == "pool":
            nc.gpsimd.tensor_copy(dst01[0:m], ps[0:m, 0:2, 0:w01])
            nc.gpsimd.tensor_copy(dst2[0:m], ps[0:m, 2, 0:w2])
        else:
            nc.vector.tensor_copy(dst01[0:m], ps[0:m, 0:2, 0:w01])
            nc.vector.tensor_copy(dst2[0:m], ps[0:m, 2, 0:w2])

    # ---- load constants ----
    def load(pool, name, shape, dt, src, tag=None):
        t = pool.tile(shape, dt, tag=tag or name, name=name)
        nc.sync.dma_start(out=t, in_=src)
        return t

    xpb = big.tile([128, NCH, AW], BF16, tag="bigA", name="xpb")
    onesk_t = load(consts, "onesk", [128, 1], BF16, onesk_d)
    for c in range(NCH):
        nc.sync.dma_start(out=xpb[:, c, 0:A], in_=xp_d[c])
    nc.vector.memset(xpb[:, :, A:AW], 0.0)
    wq_t = consts.tile([128, NCH, 2304], BF16, tag="wq", name="wq_t")
    wp_t = consts.tile([128, NCH, 768], BF16, tag="wp", name="wp_t")
    wf_t = consts.tile([128, NCH, 256], BF16, tag="wf", name="wf_t")
    selqk_t = consts.tile([128, NCH, 248], BF16, tag="selqk", name="selqk_t")
    for c in range(NCH):
        nc.sync.dma_start(out=wq_t[:, c, :], in_=wq_d[c])
        nc.sync.dma_start(out=wp_t[:, c, :], in_=wp_d[c])
        nc.sync.dma_start(out=wf_t[:, c, :], in_=wf_d[c])
        nc.sync.dma_start(out=selqk_t[:, c, :], in_=selqk_d[c])
    gsel0_t = load(consts, "gsel0", [128, 72], BF16, gsel0_d)
    gsel1_t = load(consts, "gsel1", [72, 72], BF16, gsel1_d)
    wsel0_t = load(consts, "wsel0", [128, 9, 72], BF16, wsel0_d)
    wsel1_t = load(consts, "wsel1", [72, 9, 72], BF16, wsel1_d)
    repsel9_t = load(consts, "repsel9", [72, 72], BF16, repsel9_d)
    csel_t = load(consts, "csel", [72, 128], BF16, csel_d)
    ident_t = load(consts, "ident", [128, 128], BF16, ident_d)
    bqkv_t = load(small, "bqkv", [128, 18], F32, bqkv_d)
    bproj_t = load(small, "bproj", [128, NCH], F32, bproj_d)
    bfc_t = load(small, "bfc", [128, 2], F32, bfc_d)

    # =================== box filter on Pool (residual t_mean, x9) ============
    t9 = big.tile([128, NCH, NW], BF16, tag="t9", name="t9")
    for c in range(NCH):
        tr = brp.tile([128, 1158], BF16, tag="wr", name="tr")
        nc.gpsimd.tensor_tensor(tr, xpb[:, c, 0:1158], xpb[:, c, 1:1159],
                                ALU.add)
        nc.gpsimd.tensor_tensor(tr, tr, xpb[:, c, 2:1160], ALU.add)
        nc.gpsimd.tensor_tensor(t9[:, c, :], tr[:, 0:NW], tr[:, 34:34 + NW],
                                ALU.add)
        nc.gpsimd.tensor_tensor(t9[:, c, :], t9[:, c, :], tr[:, 68:68 + NW],
                                ALU.add)

    # =================== LayerNorm 1 (stats over channels via PE) ============
    stat1 = psa()   # sum x   [1, a]
    stat2 = psb()   # sum x^2 [1, a]
    for c in range(NCH):
        sqx = prodp.tile([128, A], BF16, tag="pr", name="sqx")
        nc.vector.tensor_tensor(sqx, xpb[:, c, 0:A], xpb[:, c, 0:A], ALU.mult)
        for s, (off, w) in enumerate(SEG_A):
            nc.tensor.matmul(stat1[0:1, s, 0:w], onesk_t,
                             xpb[:, c, off:off + w],
                             start=(c == 0), stop=(c == NCH - 1),
                             skip_group_check=True)
            nc.tensor.matmul(stat2[0:1, s, 0:w], onesk_t,
                             sqx[:, off:off + w],
                             start=(c == 0), stop=(c == NCH - 1),
                             skip_group_check=True)

    def ln_smalls(stat1, stat2, width, segs, tagpfx):
        """From PSUM sums -> rstd (bf16) and -mu*rstd (bf16), [1, width]."""
        ta = small.tile([1, width], F32, tag="lnta", name=tagpfx + "ta")
        xs = small.tile([1, width], F32, tag="lnxs", name=tagpfx + "xs")
        sq = small.tile([1, width], F32, tag="lnta", name=tagpfx + "sv")
        rstd = small.tile([1, width], BF16, tag="lnrs", name=tagpfx + "rs")
        nmur = small.tile([1, width], BF16, tag="lnnm", name=tagpfx + "nm")
        eps_t = small.tile([1, 1], F32, tag="lnep", name=tagpfx + "ep")
        nc.vector.memset(eps_t, EPS)
        for s, (off, w) in enumerate(segs):
            nc.scalar.activation(ta[:, off:off + w], stat1[0:1, s, 0:w],
                                 AF.Square)
            # xs = ta/768 - stat2   (= -768*var)
            nc.vector.scalar_tensor_tensor(xs[:, off:off + w],
                                           ta[:, off:off + w], 1.0 / C,
                                           stat2[0:1, s, 0:w],
                                           ALU.mult, ALU.subtract)
        # sq = sqrt(xs * (-1/768) + eps) = sqrt(var + eps)
        nc.scalar.activation(sq, xs, AF.Sqrt, bias=eps_t, scale=-1.0 / C)
        nc.vector.reciprocal(rstd, sq)
        # nmur = (stat1 * -1/768) * rstd = -mu * rstd  (stat1 still live: the
        # next user of its PSUM tile is WAR-fenced behind this read)
        for s, (off, w) in enumerate(segs):
            nc.vector.scalar_tensor_tensor(nmur[:, off:off + w],
                                           stat1[0:1, s, 0:w], -1.0 / C,
                                           rstd[:, off:off + w],
                                           ALU.mult, ALU.mult)
        return rstd, nmur

    rstd1, nmur1 = ln_smalls(stat1, stat2, A, SEG_A, "l1")

    # broadcast rstd / nmur to 128 partitions via partition-step-0 DMA
    rrep1 = small.tile([128, A], BF16, tag="lnrr", name="rrep1")
    nrep1 = small.tile([128, A], BF16, tag="lnnr", name="nrep1")
    rscr1 = drp.tile([1, A], BF16, tag="scr", name="rscr1")
    nscr1 = drp.tile([1, A], BF16, tag="scr", name="nscr1")
    nc.sync.dma_start(out=rscr1, in_=rstd1)
    nc.sync.dma_start(out=nscr1, in_=nmur1)
    nc.sync.dma_start(out=rrep1, in_=rscr1.to_broadcast([128, A]))
    nc.sync.dma_start(out=nrep1, in_=nscr1.to_broadcast([128, A]))

    ln_b = big.tile([128, NCH, A], BF16, tag="ln", name="ln_b")
    for c in range(NCH):
        t1 = prodp.tile([128, A], BF16, tag="pr", name="t1")
        nc.vector.tensor_tensor(t1, xpb[:, c, 0:A], rrep1, ALU.mult)
        nc.vector.tensor_tensor(ln_b[:, c, :], t1, nrep1, ALU.add)

    # =================== qkv projection ======================================
    qp = big.tile([128, NCH, AW], BF16, tag="qo", name="qp")
    kp = big.tile([128, NCH, KW], BF16, tag="kp", name="kp")
    vp = big.tile([128, NCH, AW], BF16, tag="vp", name="vp")
    nc.vector.memset(qp[:, :, A:AW], 0.0)
    nc.vector.memset(kp[:, :, 0:70], 0.0)
    nc.vector.memset(kp[:, :, 70 + A:KW], 0.0)

    for g in range(18):
        ps = psa() if g % 2 == 0 else psb()
        for s, (off, w) in enumerate(SEG_A):
            for c in range(NCH):
                nc.tensor.matmul(ps[:, s, 0:w],
                                 wq_t[:, c, 128 * g:128 * (g + 1)],
                                 ln_b[:, c, off:off + w],
                                 start=(c == 0), stop=(c == NCH - 1))
        if g < 6:
            dst = qp[:, g, :]
        elif g < 12:
            dst = kp[:, g - 6, 70:]
        else:
            dst = vp[:, g - 12, :]
        copy2("act", dst[:, 0:772], dst[:, 772:1156], ps, SEG_A,
              func=AF.Identity, bias=bqkv_t[:, g:g + 1])
    # vp pad must be zero for shifted o-products
    nc.vector.memset(vp[:, :, A:AW], 0.0)

    # =================== scores: 25 displacement maps ========================
    E0 = psa()                      # [(16e x 8h), a]
    E1 = psb()                      # [(9e x 8h), a]
    for c in range(NCH):
        for ei, (er, ec) in enumerate(E_LIST):
            grp, j = (0, ei) if ei < 16 else (1, ei - 16)
            koff = 70 + 34 * er + ec
            Eg = E0 if grp == 0 else E1
            m = 128 if grp == 0 else 72
            prod = prodp.tile([128, A], BF16, tag="pr", name="prod")
            if ei in POOL_EI:
                nc.gpsimd.tensor_tensor(prod, qp[:, c, 0:A],
                                        kp[:, c, koff:koff + A], ALU.mult)
            else:
                nc.vector.tensor_tensor(prod, qp[:, c, 0:A],
                                        kp[:, c, koff:koff + A], ALU.mult)
            lhs = selqk_t[:, c, 120 - 8 * j:120 - 8 * j + m]
            first = (c == 0 and j == 0)
            last = (c == NCH - 1 and j == (15 if grp == 0 else 8))
            for s, (off, w) in enumerate(SEG_A):
                nc.tensor.matmul(Eg[0:m, s, 0:w], lhs, prod[:, off:off + w],
                                 start=first, stop=last, skip_group_check=True)

    F0 = big.tile([128, AW], BF16, tag="F0", name="F0")
    F1 = big.tile([72, AW], BF16, tag="F1", name="F1")
    nc.vector.memset(F0[:, A:AW], 0.0)
    nc.vector.memset(F1[:, A:AW], 0.0)
    for Ft, Eg, m in ((F0, E0, 128), (F1, E1, 72)):
        copy2("act", Ft[:, 0:772], Ft[:, 772:1156], Eg, SEG_A, m=m,
              func=AF.Exp, scale=SCALE)

    # =================== denominators: G (x9) -> R = 1/(9G) ==================
    Gp = psa()
    for s, (off, w) in enumerate(SEG_A):
        nc.tensor.matmul(Gp[0:72, s, 0:w], gsel0_t, F0[:, off:off + w],
                         start=True, stop=False)
        nc.tensor.matmul(Gp[0:72, s, 0:w], gsel1_t, F1[0:72, off:off + w],
                         start=False, stop=True, skip_group_check=True)
    R9all = consts.tile([72, 3, AW], BF16, tag="wq", name="R9all")
    R = big.tile([72, AW], BF16, tag="R", name="R")
    nc.vector.memset(R[:, A:AW], 0.0)
    nc.vector.memset(R9all[:, :, A:AW], 0.0)
    nc.vector.reciprocal(R[:, 0:772], Gp[0:72, 0:2, 0:386])
    nc.vector.reciprocal(R[:, 772:1156], Gp[0:72, 2, 0:384])
    R9s = [R9all[:, t, :] for t in range(3)]
    for ki in range(9):
        t, g = divmod(ki, 3)
        nc.sync.dma_start(out=R9s[t][32 * g:32 * g + 8, :],
                          in_=R[8 * ki:8 * ki + 8, :])

    # =================== W = sum_ki (wsel_ki @ F)(shift ski) * R_rep =========
    W_acc = big.tile([72, NW], BF16, tag="Wa", name="W_acc")
    for ki, (kir, kic) in enumerate(KI_LIST):
        t, gg = divmod(ki, 3)
        ski = 34 * kir + kic
        Bp = psa()   # B_ki = wsel @ F(shift)
        Rp = psb()   # rrep' = R row (shift) broadcast to 72 rows
        for s, (off, w) in enumerate(SEG_N):
            nc.tensor.matmul(Bp[0:72, s, 0:w], wsel0_t[:, ki, :],
                             F0[:, ski + off:ski + off + w],
                             start=True, stop=False)
            nc.tensor.matmul(Bp[0:72, s, 0:w], wsel1_t[0:72, ki, :],
                             F1[0:72, ski + off:ski + off + w],
                             start=False, stop=True, skip_group_check=True)
            nc.tensor.matmul(Rp[0:72, s, 0:w],
                             repsel9_t[32 * gg:32 * gg + 8, 0:72],
                             R9s[t][32 * gg:32 * gg + 8,
                                    ski + off:ski + off + w],
                             start=True, stop=True)
        B_b = brp.tile([72, NW], BF16, tag="bb", name="B_b")
        copy2("act", B_b[:, 0:726], B_b[:, 726:1088], Bp, SEG_N, m=72)
        # (B * 1.0) * rrep'  -- STT reads rrep' straight from PSUM
        dst = W_acc if ki == 0 else brp.tile([72, NW], BF16, tag="wt",
                                             name="W_tmp")
        nc.vector.scalar_tensor_tensor(dst[:, 0:726], B_b[:, 0:726], 1.0,
                                       Rp[0:72, 0:2, 0:363],
                                       ALU.mult, ALU.mult)
        nc.vector.scalar_tensor_tensor(dst[:, 726:1088], B_b[:, 726:1088],
                                       1.0, Rp[0:72, 2, 0:362],
                                       ALU.mult, ALU.mult)
        if ki > 0:
            nc.vector.tensor_tensor(W_acc, W_acc, dst, ALU.add)

    W9all = big.tile([72, 3, NW], BF16, tag="W9", name="W9all")
    W9s = [W9all[:, t, :] for t in range(3)]
    for kj in range(9):
        t, g = divmod(kj, 3)
        nc.sync.dma_start(out=W9s[t][32 * g:32 * g + 8, :],
                          in_=W_acc[8 * kj:8 * kj + 8, :])

    # =================== o_mean accumulation ================================
    # v channels are 16-interleaved (head = p//16 in every chunk), so one
    # W-broadcast per kj serves all 6 chunks; materialize all 9 upfront.
    wr_all = big.tile([128, 9, NW], BF16, tag="kp", name="wr_all")
    for kj in range(9):
        t, gg = divmod(kj, 3)
        wps = psa() if kj % 2 == 0 else psb()
        for s_, (off, w) in enumerate(SEG_N):
            nc.tensor.matmul(wps[:, s_, 0:w],
                             csel_t[32 * gg:32 * gg + 8, :],
                             W9s[t][32 * gg:32 * gg + 8, off:off + w],
                             start=True, stop=True)
        copy2("act", wr_all[:, kj, 0:726], wr_all[:, kj, 726:1088], wps, SEG_N)
    o_b = big.tile([128, NCH, NW], BF16, tag="qo", name="o_b")
    for c in range(NCH):
        oacc = psa() if c % 2 == 0 else psb()
        for kj, (kjr, kjc) in enumerate(KI_LIST):
            skj = 34 * kjr + kjc
            prod = prodp.tile([128, NW], BF16, tag="pr", name="prodo")
            nc.vector.tensor_tensor(prod, vp[:, c, skj:skj + NW],
                                    wr_all[:, kj, :], ALU.mult)
            for s_, (off, w) in enumerate(SEG_N):
                nc.tensor.matmul(oacc[:, s_, 0:w], ident_t,
                                 prod[:, off:off + w],
                                 start=(kj == 0), stop=(kj == 8),
                                 skip_group_check=True)
        copy2("act", o_b[:, c, 0:726], o_b[:, c, 726:1088], oacc, SEG_N)

    # =================== proj + residual -> u ================================
    u_b = big.tile([128, NCH, NW], BF16, tag="ub", name="u_b")
    for g in range(NCH):
        ps = psa() if g % 2 == 0 else psb()
        for s, (off, w) in enumerate(SEG_N):
            for c in range(NCH):
                nc.tensor.matmul(ps[:, s, 0:w],
                                 wp_t[:, c, 128 * g:128 * (g + 1)],
                                 o_b[:, c, off:off + w],
                                 start=(c == 0), stop=(c == NCH - 1))
        # u = t9/9 + r, then + bproj
        nc.vector.scalar_tensor_tensor(u_b[:, g, 0:726], t9[:, g, 0:726],
                                       1.0 / 9.0, ps[:, 0:2, 0:363],
                                       ALU.mult, ALU.add)
        nc.vector.scalar_tensor_tensor(u_b[:, g, 726:1088], t9[:, g, 726:1088],
                                       1.0 / 9.0, ps[:, 2, 0:362],
                                       ALU.mult, ALU.add)
        nc.vector.tensor_scalar_add(u_b[:, g, :], u_b[:, g, :],
                                    bproj_t[:, g:g + 1])

    # =================== LayerNorm 2 ========================================
    stat1b = psa()
    stat2b = psb()
    for c in range(NCH):
        sq2 = prodp.tile([128, NW], BF16, tag="pr", name="sq2")
        nc.vector.tensor_tensor(sq2, u_b[:, c, :], u_b[:, c, :], ALU.mult)
        for s, (off, w) in enumerate(SEG_N):
            nc.tensor.matmul(stat1b[0:1, s, 0:w], onesk_t,
                             u_b[:, c, off:off + w],
                             start=(c == 0), stop=(c == NCH - 1),
                             skip_group_check=True)
            nc.tensor.matmul(stat2b[0:1, s, 0:w], onesk_t,
                             sq2[:, off:off + w],
                             start=(c == 0), stop=(c == NCH - 1),
                             skip_group_check=True)
    rstd2, nmur2 = ln_smalls(stat1b, stat2b, NW, SEG_N, "l2")
    rrep2 = small.tile([128, NW], BF16, tag="lnrr", name="rrep2")
    nrep2 = small.tile([128, NW], BF16, tag="lnnr", name="nrep2")
    rscr2 = drp.tile([1, NW], BF16, tag="scr", name="rscr2")
    nscr2 = drp.tile([1, NW], BF16, tag="scr", name="nscr2")
    nc.sync.dma_start(out=rscr2, in_=rstd2)
    nc.sync.dma_start(out=nscr2, in_=nmur2)
    nc.sync.dma_start(out=rrep2, in_=rscr2.to_broadcast([128, NW]))
    nc.sync.dma_start(out=nrep2, in_=nscr2.to_broadcast([128, NW]))
    ln2_b = big.tile([128, NCH, NW], BF16, tag="ln", name="ln2_b")
    for c in range(NCH):
        t1 = prodp.tile([128, NW], BF16, tag="pr", name="t1b")
        nc.vector.tensor_tensor(t1, u_b[:, c, :], rrep2, ALU.mult)
        nc.vector.tensor_tensor(ln2_b[:, c, :], t1, nrep2, ALU.add)

    # =================== fc + relu + output =================================
    y_t = big.tile([128, 2, NW], F32, tag="kp", name="y_t")
    for g in range(2):
        ps = psa() if g % 2 == 0 else psb()
        for s, (off, w) in enumerate(SEG_N):
            for c in range(NCH):
                nc.tensor.matmul(ps[:, s, 0:w],
                                 wf_t[:, c, 128 * g:128 * (g + 1)],
                                 ln2_b[:, c, off:off + w],
                                 start=(c == 0), stop=(c == NCH - 1))
        copy2("act", y_t[:, g, 0:726], y_t[:, g, 726:1088], ps, SEG_N,
              func=AF.Relu, bias=bfc_t[:, g:g + 1])
    for g in range(2):
        src = y_t[:, g, :].rearrange("p (r c) -> p r c", c=34)[:, :, 0:32]
        nc.sync.dma_start(out=y_d[g], in_=src)


# ============================ host-side wrapper =============================

def _build_sels():
    bf = ml_dtypes.bfloat16
    selqk = np.zeros((NCH, 128, 248), np.float32)
    for c in range(NCH):
        for r in range(128):
            h = (128 * c + r) // HD
            selqk[c, r, 120 + h] = 1.0
    gsel0 = np.zeros((128, 72), np.float32)
    gsel1 = np.zeros((72, 72), np.float32)
    for ki, (kir, kic) in enumerate(KI_LIST):
        for j, (er, ec) in enumerate(E_LIST):
            if -kir <= er <= 2 - kir and -kic <= ec <= 2 - kic:
                for h in range(HEADS):
                    if j < 16:
                        gsel0[8 * j + h, 8 * ki + h] = 9.0
                    else:
                        gsel1[8 * (j - 16) + h, 8 * ki + h] = 9.0
    wsel0 = np.zeros((128, 9, 72), np.float32)
    wsel1 = np.zeros((72, 9, 72), np.float32)
    for ki, (kir, kic) in enumerate(KI_LIST):
        for j, (er, ec) in enumerate(E_LIST):
            kjr, kjc = er + kir, ec + kic
            if 0 <= kjr <= 2 and 0 <= kjc <= 2:
                kj = 3 * kjr + kjc
                for h in range(HEADS):
                    if j < 16:
                        wsel0[8 * j + h, ki, 8 * kj + h] = 1.0
                    else:
                        wsel1[8 * (j - 16) + h, ki, 8 * kj + h] = 1.0
    repsel9 = np.zeros((72, 72), np.float32)
    csel = np.zeros((72, 128), np.float32)
    for g in range(3):
        for h in range(HEADS):
            for j in range(9):
                repsel9[32 * g + h, 8 * j + h] = 1.0
        for p in range(128):
            csel[32 * g + p // 16, p] = 1.0
    ident = np.eye(128, dtype=np.float32)
    onesk = np.ones((128, 1), np.float32)
    out = dict(selqk=selqk, gsel0=gsel0, gsel1=gsel1, wsel0=wsel0, wsel1=wsel1,
               repsel9=repsel9, csel=csel, ident=ident, onesk=onesk)
    return {k: v.astype(bf) for k, v in out.items()}


@functools.lru_cache(maxsize=1)
def _build_module():
    nc = bacc.Bacc("TRN2", target_bir_lowering=False, debug=False)
    ins = {}

    def din(name, shape, dt):
        ins[name] = nc.dram_tensor(name, shape, dt, kind="ExternalInput").ap()

    din("xp", [NCH, 128, A], BF16)
    din("wqkv", [NCH, 128, 2304], BF16)
    din("wproj", [NCH, 128, 768], BF16)
    din("wfc", [NCH, 128, 256], BF16)
    din("bqkv", [128, 18], F32)
    din("bproj", [128, NCH], F32)
    din("bfc", [128, 2], F32)
    din("selqk", [NCH, 128, 248], BF16)
    din("gsel0", [128, 72], BF16)
    din("gsel1", [72, 72], BF16)
    din("wsel0", [128, 9, 72], BF16)
    din("wsel1", [72, 9, 72], BF16)
    din("repsel9", [72, 72], BF16)
    din("csel", [72, 128], BF16)
    din("ident", [128, 128], BF16)
    din("onesk", [128, 1], BF16)
    outs = {"y": nc.dram_tensor("y", [2, 128, 32, 32], F32,
                                kind="ExternalOutput").ap()}

    from contextlib import ExitStack
    with tile.TileContext(nc) as tc:
        with ExitStack() as ctx:
            with nc.allow_low_precision(reason="bf16 kernel by design"):
                emit_kernel(ctx, tc, ins, outs)
    nc.compile()
    return nc


def kernel(x, w_qkv, b_qkv, w_proj, b_proj, g1, beta1, g2, beta2, w_fc, b_fc,
           _run_kwargs=None):
    bf = ml_dtypes.bfloat16
    x = np.asarray(x, np.float32)
    B = x.shape[0]
    assert x.shape == (8, C, 32, 32)

    w_qkv = np.asarray(w_qkv, np.float32)
    b_qkv = np.asarray(b_qkv, np.float32)
    w_fc = np.asarray(w_fc, np.float32)
    b_fc = np.asarray(b_fc, np.float32)
    g1 = np.asarray(g1, np.float32)
    beta1 = np.asarray(beta1, np.float32)
    g2 = np.asarray(g2, np.float32)
    beta2 = np.asarray(beta2, np.float32)
    wq = g1[:, None] * w_qkv
    bq = beta1 @ w_qkv + b_qkv
    # 16-interleave v out-channels (head = p//16 within every 128-chunk) and
    # permute proj rows to match
    old_of_new = np.array([96 * ((n % 128) // 16) + 16 * (n // 128)
                           + (n % 16) for n in range(C)])
    wq[:, 1536:] = wq[:, 1536 + old_of_new]
    bq[1536:] = bq[1536 + old_of_new]
    w_proj = np.asarray(w_proj, np.float32)[old_of_new, :]
    wf = g2[:, None] * w_fc
    bfc2 = beta2 @ w_fc + b_fc

    sels = _build_sels()
    shared = dict(
        wqkv=np.ascontiguousarray(wq.reshape(NCH, 128, 2304)).astype(bf),
        wproj=np.ascontiguousarray(
            np.asarray(w_proj, np.float32).reshape(NCH, 128, 768)).astype(bf),
        wfc=np.ascontiguousarray(wf.reshape(NCH, 128, 256)).astype(bf),
        bqkv=np.ascontiguousarray(bq.reshape(18, 128).T),
        bproj=np.ascontiguousarray(
            np.asarray(b_proj, np.float32).reshape(NCH, 128).T),
        bfc=np.ascontiguousarray(bfc2.reshape(2, 128).T),
        **sels,
    )
    in_maps = []
    for b in range(B):
        xpad = np.pad(x[b], ((0, 0), (1, 1), (1, 1)), mode="edge")
        xp = np.ascontiguousarray(xpad.reshape(NCH, 128, A)).astype(bf)
        in_maps.append(dict(xp=xp, **shared))

    nc = _build_module()
    res = run_bass_kernel_spmd(nc, in_maps, core_ids=list(range(8)),
                               **(_run_kwargs or {}))
    outs = []
    for b in range(B):
        y = np.asarray(res.results[b]["y"], np.float32)  # [2,128,32,32]
        outs.append(y.reshape(256, 32, 32))
    out = np.stack(outs).astype(np.float32)
    if _run_kwargs is not None:
        kernel.last_result = res
    return out


# revision 4
# speedup vs baseline: 1.0175x; 1.0175x over previous
"""Trainium2 Bass kernel for nn_AttnBlock (sparse 3x3-window attention block).

Structure (per core, one batch image):
  - LN1/qkv computed once per padded pixel (34x34 grid); g1/beta1 folded into
    w_qkv/b_qkv and g2/beta2 into w_fc/b_fc on the host, so LN applies are two
    plain DVE ops.
  - Scores deduplicated into 25 displacement maps E_e = q . shift_e(k)
    (DVE/Pool elementwise product + PE per-head column reduction).
  - Softmax denominators via box-sum selection matmul (gsel, scaled x9 to fold
    the window mean), W = sum_ki (wsel_ki @ F)(shift) * R-broadcast.
  - o_mean = sum_kj shift(v * W-broadcast); broadcasts via PE selection
    matmuls, PSUM->SBUF bf16 copies on Act/Pool so big DVE products run in
    2x (16-bit) mode, full width.
  - Box filter (residual mean) runs on the Pool engine under the qkv phase.

Sharding: data-parallel over batch B=8 -> one batch per NeuronCore.
"""

import functools
import numpy as np
import ml_dtypes

import concourse.bass as bass
import concourse.mybir as mybir
import concourse.tile as tile
from concourse import bacc
from concourse.bass_utils import run_bass_kernel_spmd

F32 = mybir.dt.float32
BF16 = mybir.dt.bfloat16
AF = mybir.ActivationFunctionType
ALU = mybir.AluOpType

C = 768
NCH = 6          # channel chunks of 128
G = 34           # padded grid side
A = G * G        # 1156 padded pixels
AW = 1160        # padded-pixel width with 4 pad cols
NW = 1088        # window-grid width = 32*34 (rows 0..31, cols 0..33)
KW = 1300        # k map width with +-70 margins (content at 70)
HEADS = 8
HD = 96
SCALE = HD ** -0.5
EPS = 1e-5

SEG_A = [(0, 386), (386, 386), (772, 384)]   # PSUM segs over the a-grid
SEG_N = [(0, 363), (363, 363), (726, 362)]   # PSUM segs over the n-grid

E_LIST = [(er, ec) for er in range(-2, 3) for ec in range(-2, 3)]  # 25
KI_LIST = [(r, c) for r in range(3) for c in range(3)]             # 9

POOL_EI = {4, 9, 14, 19, 24}  # displacement products offloaded to Pool


def emit_kernel(ctx, tc, ins, outs):
    nc = tc.nc
    xp_d = ins["xp"]          # [6,128,1156] bf16
    wq_d = ins["wqkv"]        # [6,128,2304] bf16 (g1-folded)
    wp_d = ins["wproj"]       # [6,128,768] bf16
    wf_d = ins["wfc"]         # [6,128,256] bf16 (g2-folded)
    bqkv_d = ins["bqkv"]      # [128,18] f32 (beta1-folded)
    bproj_d = ins["bproj"]    # [128,6] f32
    bfc_d = ins["bfc"]        # [128,2] f32 (beta2-folded)
    selqk_d = ins["selqk"]    # [6,128,248] bf16
    gsel0_d = ins["gsel0"]    # [128,72] bf16 (x9)
    gsel1_d = ins["gsel1"]    # [72,72] bf16 (x9)
    wsel0_d = ins["wsel0"]    # [128,9,72] bf16
    wsel1_d = ins["wsel1"]    # [72,9,72] bf16
    repsel9_d = ins["repsel9"]  # [72,72] bf16
    csel_d = ins["csel"]      # [72,128] bf16
    ident_d = ins["ident"]    # [128,128] bf16
    onesk_d = ins["onesk"]    # [128,1] bf16
    y_d = outs["y"]           # [2,128,32,32] f32

    consts = ctx.enter_context(tc.tile_pool(name="consts", bufs=1))
    big = ctx.enter_context(tc.tile_pool(name="big", bufs=1))
    prodp = ctx.enter_context(tc.tile_pool(name="prodp", bufs=5))
    brp = ctx.enter_context(tc.tile_pool(name="brp", bufs=2))
    small = ctx.enter_context(tc.tile_pool(name="small", bufs=1))
    psA = ctx.enter_context(tc.tile_pool(name="psA", bufs=1, space="PSUM"))
    psB = ctx.enter_context(tc.tile_pool(name="psB", bufs=1, space="PSUM"))
    psC = ctx.enter_context(tc.tile_pool(name="psC", bufs=2, space="PSUM"))
    drp = ctx.enter_context(tc.tile_pool(name="drp", bufs=2, space="DRAM"))

    def psa():
        return psA.tile([128, 3, 512], F32, tag="a", name="psa_t")

    def psb():
        return psB.tile([128, 3, 512], F32, tag="b", name="psb_t")

    # two-op PSUM->SBUF copy helpers (banks 0-1 fused, bank 2), SEG-uniform
    def copy2(eng, dst01, dst2, ps, segs, m=128, func=AF.Copy, bias=None,
              scale=1.0):
        w01, w2 = segs[0][1], segs[2][1]
        if eng == "act":
            kw = {} if bias is None else {"bias": bias}
            nc.scalar.activation(dst01[0:m], ps[0:m, 0:2, 0:w01], func,
                                 scale=scale, **kw)
            nc.scalar.activation(dst2[0:m], ps[0:m, 2, 0:w2], func,
                                 scale=scale, **kw)
        elif eng == "pool":
            nc.gpsimd.tensor_copy(dst01[0:m], ps[0:m, 0:2, 0:w01])
            nc.gpsimd.tensor_copy(dst2[0:m], ps[0:m, 2, 0:w2])
        else:
            nc.vector.tensor_copy(dst01[0:m], ps[0:m, 0:2, 0:w01])
            nc.vector.tensor_copy(dst2[0:m], ps[0:m, 2, 0:w2])

    # ---- load constants ----
    def load(pool, name, shape, dt, src, tag=None):
        t = pool.tile(shape, dt, tag=tag or name, name=name)
        nc.sync.dma_start(out=t, in_=src)
        return t

    xpb = big.tile([128, NCH, AW], BF16, tag="bigA", name="xpb")
    onesk_t = load(consts, "onesk", [128, 1], BF16, onesk_d)
    for c in range(NCH):
        nc.sync.dma_start(out=xpb[:, c, 0:A], in_=xp_d[c])
    nc.vector.memset(xpb[:, :, A:AW], 0.0)
    wq_t = consts.tile([128, NCH, 2304], BF16, tag="wq", name="wq_t")
    wp_t = consts.tile([128, NCH, 768], BF16, tag="wp", name="wp_t")
    wf_t = consts.tile([128, NCH, 256], BF16, tag="wf", name="wf_t")
    selqk_t = consts.tile([128, NCH, 248], BF16, tag="selqk", name="selqk_t")
    for c in range(NCH):
        nc.sync.dma_start(out=wq_t[:, c, :], in_=wq_d[c])
        nc.sync.dma_start(out=wp_t[:, c, :], in_=wp_d[c])
        nc.sync.dma_start(out=wf_t[:, c, :], in_=wf_d[c])
        nc.sync.dma_start(out=selqk_t[:, c, :], in_=selqk_d[c])
    gsel0_t = load(consts, "gsel0", [128, 72], BF16, gsel0_d)
    gsel1_t = load(consts, "gsel1", [72, 72], BF16, gsel1_d)
    wsel0_t = load(consts, "wsel0", [128, 9, 72], BF16, wsel0_d)
    wsel1_t = load(consts, "wsel1", [72, 9, 72], BF16, wsel1_d)
    repsel9_t = load(consts, "repsel9", [72, 72], BF16, repsel9_d)
    csel_t = load(consts, "csel", [72, 128], BF16, csel_d)
    ident_t = load(consts, "ident", [128, 128], BF16, ident_d)
    bqkv_t = load(small, "bqkv", [128, 18], F32, bqkv_d)
    bproj_t = load(small, "bproj", [128, NCH], F32, bproj_d)
    bfc_t = load(small, "bfc", [128, 2], F32, bfc_d)

    # =================== box filter on Pool (residual t_mean, x9) ============
    t9 = big.tile([128, NCH, NW], BF16, tag="t9", name="t9")
    for c in range(NCH):
        tr = brp.tile([128, 1158], BF16, tag="wr", name="tr")
        nc.gpsimd.tensor_tensor(tr, xpb[:, c, 0:1158], xpb[:, c, 1:1159],
                                ALU.add)
        nc.gpsimd.tensor_tensor(tr, tr, xpb[:, c, 2:1160], ALU.add)
        nc.gpsimd.tensor_tensor(t9[:, c, :], tr[:, 0:NW], tr[:, 34:34 + NW],
                                ALU.add)
        nc.gpsimd.tensor_tensor(t9[:, c, :], t9[:, c, :], tr[:, 68:68 + NW],
                                ALU.add)

    # =================== LayerNorm 1 (stats over channels via PE) ============
    stat1 = psa()   # sum x   [1, a]
    stat2 = psb()   # sum x^2 [1, a]
    for c in range(NCH):
        sqx = prodp.tile([128, A], BF16, tag="pr", name="sqx")
        nc.vector.tensor_tensor(sqx, xpb[:, c, 0:A], xpb[:, c, 0:A], ALU.mult)
        for s, (off, w) in enumerate(SEG_A):
            nc.tensor.matmul(stat1[0:1, s, 0:w], onesk_t,
                             xpb[:, c, off:off + w],
                             start=(c == 0), stop=(c == NCH - 1),
                             skip_group_check=True)
            nc.tensor.matmul(stat2[0:1, s, 0:w], onesk_t,
                             sqx[:, off:off + w],
                             start=(c == 0), stop=(c == NCH - 1),
                             skip_group_check=True)

    def ln_smalls(stat1, stat2, width, segs, tagpfx):
        """From PSUM sums -> rstd (bf16) and -mu*rstd (bf16), [1, width]."""
        ta = small.tile([1, width], F32, tag="lnta", name=tagpfx + "ta")
        xs = small.tile([1, width], F32, tag="lnxs", name=tagpfx + "xs")
        sq = small.tile([1, width], F32, tag="lnta", name=tagpfx + "sv")
        rstd = small.tile([1, width], BF16, tag="lnrs", name=tagpfx + "rs")
        nmur = small.tile([1, width], BF16, tag="lnnm", name=tagpfx + "nm")
        eps_t = small.tile([1, 1], F32, tag="lnep", name=tagpfx + "ep")
        nc.vector.memset(eps_t, EPS)
        for s, (off, w) in enumerate(segs):
            nc.scalar.activation(ta[:, off:off + w], stat1[0:1, s, 0:w],
                                 AF.Square)
            # xs = ta/768 - stat2   (= -768*var)
            nc.vector.scalar_tensor_tensor(xs[:, off:off + w],
                                           ta[:, off:off + w], 1.0 / C,
                                           stat2[0:1, s, 0:w],
                                           ALU.mult, ALU.subtract)
        # sq = sqrt(xs * (-1/768) + eps) = sqrt(var + eps)
        nc.scalar.activation(sq, xs, AF.Sqrt, bias=eps_t, scale=-1.0 / C)
        nc.vector.reciprocal(rstd, sq)
        # nmur = (stat1 * -1/768) * rstd = -mu * rstd  (stat1 still live: the
        # next user of its PSUM tile is WAR-fenced behind this read)
        for s, (off, w) in enumerate(segs):
            nc.vector.scalar_tensor_tensor(nmur[:, off:off + w],
                                           stat1[0:1, s, 0:w], -1.0 / C,
                                           rstd[:, off:off + w],
                                           ALU.mult, ALU.mult)
        return rstd, nmur

    rstd1, nmur1 = ln_smalls(stat1, stat2, A, SEG_A, "l1")

    # broadcast rstd / nmur to 128 partitions via partition-step-0 DMA
    rrep1 = small.tile([128, A], BF16, tag="lnrr", name="rrep1")
    nrep1 = small.tile([128, A], BF16, tag="lnnr", name="nrep1")
    rscr1 = drp.tile([1, A], BF16, tag="scr", name="rscr1")
    nscr1 = drp.tile([1, A], BF16, tag="scr", name="nscr1")
    nc.sync.dma_start(out=rscr1, in_=rstd1)
    nc.sync.dma_start(out=nscr1, in_=nmur1)
    nc.sync.dma_start(out=rrep1, in_=rscr1.to_broadcast([128, A]))
    nc.sync.dma_start(out=nrep1, in_=nscr1.to_broadcast([128, A]))

    ln_b = big.tile([128, NCH, A], BF16, tag="ln", name="ln_b")
    for s_, (off, w) in enumerate(SEG_A):
        for c in range(NCH):
            t1 = prodp.tile([128, A], BF16, tag="pr", name="t1")
            nc.vector.tensor_tensor(t1[:, 0:w], xpb[:, c, off:off + w],
                                    rrep1[:, off:off + w], ALU.mult)
            nc.vector.tensor_tensor(ln_b[:, c, off:off + w], t1[:, 0:w],
                                    nrep1[:, off:off + w], ALU.add)

    # =================== qkv projection ======================================
    qp = big.tile([128, NCH, AW], BF16, tag="qo", name="qp")
    kp = big.tile([128, NCH, KW], BF16, tag="kp", name="kp")
    vp = big.tile([128, NCH, AW], BF16, tag="vp", name="vp")
    nc.vector.memset(qp[:, :, A:AW], 0.0)
    nc.vector.memset(kp[:, :, 0:70], 0.0)
    nc.vector.memset(kp[:, :, 70 + A:KW], 0.0)

    for g in range(18):
        ps = psa() if g % 2 == 0 else psb()
        for s, (off, w) in enumerate(SEG_A):
            for c in range(NCH):
                nc.tensor.matmul(ps[:, s, 0:w],
                                 wq_t[:, c, 128 * g:128 * (g + 1)],
                                 ln_b[:, c, off:off + w],
                                 start=(c == 0), stop=(c == NCH - 1))
        if g < 6:
            dst = qp[:, g, :]
        elif g < 12:
            dst = kp[:, g - 6, 70:]
        else:
            dst = vp[:, g - 12, :]
        copy2("act", dst[:, 0:772], dst[:, 772:1156], ps, SEG_A,
              func=AF.Identity, bias=bqkv_t[:, g:g + 1])
    # vp pad must be zero for shifted o-products
    nc.vector.memset(vp[:, :, A:AW], 0.0)

    # =================== scores: 25 displacement maps ========================
    E0 = psa()                      # [(16e x 8h), a]
    E1 = psb()                      # [(9e x 8h), a]
    for c in range(NCH):
        for ei, (er, ec) in enumerate(E_LIST):
            grp, j = (0, ei) if ei < 16 else (1, ei - 16)
            koff = 70 + 34 * er + ec
            Eg = E0 if grp == 0 else E1
            m = 128 if grp == 0 else 72
            prod = prodp.tile([128, A], BF16, tag="pr", name="prod")
            if ei in POOL_EI:
                nc.gpsimd.tensor_tensor(prod, qp[:, c, 0:A],
                                        kp[:, c, koff:koff + A], ALU.mult)
            else:
                nc.vector.tensor_tensor(prod, qp[:, c, 0:A],
                                        kp[:, c, koff:koff + A], ALU.mult)
            lhs = selqk_t[:, c, 120 - 8 * j:120 - 8 * j + m]
            first = (c == 0 and j == 0)
            last = (c == NCH - 1 and j == (15 if grp == 0 else 8))
            for s, (off, w) in enumerate(SEG_A):
                nc.tensor.matmul(Eg[0:m, s, 0:w], lhs, prod[:, off:off + w],
                                 start=first, stop=last, skip_group_check=True)

    F0 = big.tile([128, AW], BF16, tag="F0", name="F0")
    F1 = big.tile([72, AW], BF16, tag="F1", name="F1")
    nc.vector.memset(F0[:, A:AW], 0.0)
    nc.vector.memset(F1[:, A:AW], 0.0)
    for Ft, Eg, m in ((F0, E0, 128), (F1, E1, 72)):
        copy2("act", Ft[:, 0:772], Ft[:, 772:1156], Eg, SEG_A, m=m,
              func=AF.Exp, scale=SCALE)

    # =================== denominators: G (x9) -> R = 1/(9G) ==================
    Gp = psa()
    for s, (off, w) in enumerate(SEG_A):
        nc.tensor.matmul(Gp[0:72, s, 0:w], gsel0_t, F0[:, off:off + w],
                         start=True, stop=False)
        nc.tensor.matmul(Gp[0:72, s, 0:w], gsel1_t, F1[0:72, off:off + w],
                         start=False, stop=True, skip_group_check=True)
    R9all = consts.tile([72, 3, AW], BF16, tag="wq", name="R9all")
    R = big.tile([72, AW], BF16, tag="R", name="R")
    nc.vector.memset(R[:, A:AW], 0.0)
    nc.vector.memset(R9all[:, :, A:AW], 0.0)
    nc.vector.reciprocal(R[:, 0:772], Gp[0:72, 0:2, 0:386])
    nc.vector.reciprocal(R[:, 772:1156], Gp[0:72, 2, 0:384])
    R9s = [R9all[:, t, :] for t in range(3)]
    for ki in range(9):
        t, g = divmod(ki, 3)
        nc.sync.dma_start(out=R9s[t][32 * g:32 * g + 8, :],
                          in_=R[8 * ki:8 * ki + 8, :])

    # =================== W = sum_ki (wsel_ki @ F)(shift ski) * R_rep =========
    W_acc = big.tile([72, NW], BF16, tag="Wa", name="W_acc")
    for ki, (kir, kic) in enumerate(KI_LIST):
        t, gg = divmod(ki, 3)
        ski = 34 * kir + kic
        Bp = psa()   # B_ki = wsel @ F(shift)
        Rp = psb()   # rrep' = R row (shift) broadcast to 72 rows
        for s, (off, w) in enumerate(SEG_N):
            nc.tensor.matmul(Bp[0:72, s, 0:w], wsel0_t[:, ki, :],
                             F0[:, ski + off:ski + off + w],
                             start=True, stop=False)
            nc.tensor.matmul(Bp[0:72, s, 0:w], wsel1_t[0:72, ki, :],
                             F1[0:72, ski + off:ski + off + w],
                             start=False, stop=True, skip_group_check=True)
            nc.tensor.matmul(Rp[0:72, s, 0:w],
                             repsel9_t[32 * gg:32 * gg + 8, 0:72],
                             R9s[t][32 * gg:32 * gg + 8,
                                    ski + off:ski + off + w],
                             start=True, stop=True)
        B_b = brp.tile([72, NW], BF16, tag="bb", name="B_b")
        copy2("act", B_b[:, 0:726], B_b[:, 726:1088], Bp, SEG_N, m=72)
        # (B * 1.0) * rrep'  -- STT reads rrep' straight from PSUM
        dst = W_acc if ki == 0 else brp.tile([72, NW], BF16, tag="wt",
                                             name="W_tmp")
        nc.vector.scalar_tensor_tensor(dst[:, 0:726], B_b[:, 0:726], 1.0,
                                       Rp[0:72, 0:2, 0:363],
                                       ALU.mult, ALU.mult)
        nc.vector.scalar_tensor_tensor(dst[:, 726:1088], B_b[:, 726:1088],
                                       1.0, Rp[0:72, 2, 0:362],
                                       ALU.mult, ALU.mult)
        if ki > 0:
            nc.vector.tensor_tensor(W_acc, W_acc, dst, ALU.add)

    W9all = big.tile([72, 3, NW], BF16, tag="W9", name="W9all")
    W9s = [W9all[:, t, :] for t in range(3)]
    for kj in range(9):
        t, g = divmod(kj, 3)
        nc.sync.dma_start(out=W9s[t][32 * g:32 * g + 8, :],
                          in_=W_acc[8 * kj:8 * kj + 8, :])

    # =================== o_mean accumulation ================================
    # v channels are 16-interleaved (head = p//16 in every chunk), so one
    # W-broadcast per kj serves all 6 chunks; materialize all 9 upfront.
    wr_all = big.tile([128, 9, NW], BF16, tag="kp", name="wr_all")
    for kj in range(9):
        t, gg = divmod(kj, 3)
        wps = psa() if kj % 2 == 0 else psb()
        for s_, (off, w) in enumerate(SEG_N):
            nc.tensor.matmul(wps[:, s_, 0:w],
                             csel_t[32 * gg:32 * gg + 8, :],
                             W9s[t][32 * gg:32 * gg + 8, off:off + w],
                             start=True, stop=True)
        copy2("act", wr_all[:, kj, 0:726], wr_all[:, kj, 726:1088], wps, SEG_N)
    o_b = big.tile([128, NCH, NW], BF16, tag="qo", name="o_b")
    for c in range(NCH):
        oacc = psa() if c % 2 == 0 else psb()
        for kj, (kjr, kjc) in enumerate(KI_LIST):
            skj = 34 * kjr + kjc
            prod = prodp.tile([128, NW], BF16, tag="pr", name="prodo")
            nc.vector.tensor_tensor(prod, vp[:, c, skj:skj + NW],
                                    wr_all[:, kj, :], ALU.mult)
            for s_, (off, w) in enumerate(SEG_N):
                nc.tensor.matmul(oacc[:, s_, 0:w], ident_t,
                                 prod[:, off:off + w],
                                 start=(kj == 0), stop=(kj == 8),
                                 skip_group_check=True)
        copy2("act", o_b[:, c, 0:726], o_b[:, c, 726:1088], oacc, SEG_N)

    # =================== proj + residual -> u ================================
    u_b = big.tile([128, NCH, NW], BF16, tag="ub", name="u_b")
    for g in range(NCH):
        ps = psa() if g % 2 == 0 else psb()
        for s, (off, w) in enumerate(SEG_N):
            for c in range(NCH):
                nc.tensor.matmul(ps[:, s, 0:w],
                                 wp_t[:, c, 128 * g:128 * (g + 1)],
                                 o_b[:, c, off:off + w],
                                 start=(c == 0), stop=(c == NCH - 1))
        # u = t9/9 + r, then + bproj
        nc.vector.scalar_tensor_tensor(u_b[:, g, 0:726], t9[:, g, 0:726],
                                       1.0 / 9.0, ps[:, 0:2, 0:363],
                                       ALU.mult, ALU.add)
        nc.vector.scalar_tensor_tensor(u_b[:, g, 726:1088], t9[:, g, 726:1088],
                                       1.0 / 9.0, ps[:, 2, 0:362],
                                       ALU.mult, ALU.add)
        nc.vector.tensor_scalar_add(u_b[:, g, :], u_b[:, g, :],
                                    bproj_t[:, g:g + 1])

    # =================== LayerNorm 2 ========================================
    stat1b = psa()
    stat2b = psb()
    for c in range(NCH):
        sq2 = prodp.tile([128, NW], BF16, tag="pr", name="sq2")
        nc.vector.tensor_tensor(sq2, u_b[:, c, :], u_b[:, c, :], ALU.mult)
        for s, (off, w) in enumerate(SEG_N):
            nc.tensor.matmul(stat1b[0:1, s, 0:w], onesk_t,
                             u_b[:, c, off:off + w],
                             start=(c == 0), stop=(c == NCH - 1),
                             skip_group_check=True)
            nc.tensor.matmul(stat2b[0:1, s, 0:w], onesk_t,
                             sq2[:, off:off + w],
                             start=(c == 0), stop=(c == NCH - 1),
                             skip_group_check=True)
    rstd2, nmur2 = ln_smalls(stat1b, stat2b, NW, SEG_N, "l2")
    rrep2 = small.tile([128, NW], BF16, tag="lnrr", name="rrep2")
    nrep2 = small.tile([128, NW], BF16, tag="lnnr", name="nrep2")
    rscr2 = drp.tile([1, NW], BF16, tag="scr", name="rscr2")
    nscr2 = drp.tile([1, NW], BF16, tag="scr", name="nscr2")
    nc.sync.dma_start(out=rscr2, in_=rstd2)
    nc.sync.dma_start(out=nscr2, in_=nmur2)
    nc.sync.dma_start(out=rrep2, in_=rscr2.to_broadcast([128, NW]))
    nc.sync.dma_start(out=nrep2, in_=nscr2.to_broadcast([128, NW]))
    ln2_b = big.tile([128, NCH, NW], BF16, tag="ln", name="ln2_b")
    for s_, (off, w) in enumerate(SEG_N):
        for c in range(NCH):
            t1 = prodp.tile([128, NW], BF16, tag="pr", name="t1b")
            nc.vector.tensor_tensor(t1[:, 0:w], u_b[:, c, off:off + w],
                                    rrep2[:, off:off + w], ALU.mult)
            nc.vector.tensor_tensor(ln2_b[:, c, off:off + w], t1[:, 0:w],
                                    nrep2[:, off:off + w], ALU.add)

    # =================== fc + relu + output =================================
    y_t = big.tile([128, 2, NW], F32, tag="kp", name="y_t")
    for g in range(2):
        ps = psa() if g % 2 == 0 else psb()
        for s, (off, w) in enumerate(SEG_N):
            for c in range(NCH):
                nc.tensor.matmul(ps[:, s, 0:w],
                                 wf_t[:, c, 128 * g:128 * (g + 1)],
                                 ln2_b[:, c, off:off + w],
                                 start=(c == 0), stop=(c == NCH - 1))
        copy2("act", y_t[:, g, 0:726], y_t[:, g, 726:1088], ps, SEG_N,
              func=AF.Relu, bias=bfc_t[:, g:g + 1])
        src = y_t[:, g, :].rearrange("p (r c) -> p r c", c=34)[:, :, 0:32]
        nc.sync.dma_start(out=y_d[g], in_=src)


# ============================ host-side wrapper =============================

def _build_sels():
    bf = ml_dtypes.bfloat16
    selqk = np.zeros((NCH, 128, 248), np.float32)
    for c in range(NCH):
        for r in range(128):
            h = (128 * c + r) // HD
            selqk[c, r, 120 + h] = 1.0
    gsel0 = np.zeros((128, 72), np.float32)
    gsel1 = np.zeros((72, 72), np.float32)
    for ki, (kir, kic) in enumerate(KI_LIST):
        for j, (er, ec) in enumerate(E_LIST):
            if -kir <= er <= 2 - kir and -kic <= ec <= 2 - kic:
                for h in range(HEADS):
                    if j < 16:
                        gsel0[8 * j + h, 8 * ki + h] = 9.0
                    else:
                        gsel1[8 * (j - 16) + h, 8 * ki + h] = 9.0
    wsel0 = np.zeros((128, 9, 72), np.float32)
    wsel1 = np.zeros((72, 9, 72), np.float32)
    for ki, (kir, kic) in enumerate(KI_LIST):
        for j, (er, ec) in enumerate(E_LIST):
            kjr, kjc = er + kir, ec + kic
            if 0 <= kjr <= 2 and 0 <= kjc <= 2:
                kj = 3 * kjr + kjc
                for h in range(HEADS):
                    if j < 16:
                        wsel0[8 * j + h, ki, 8 * kj + h] = 1.0
                    else:
                        wsel1[8 * (j - 16) + h, ki, 8 * kj + h] = 1.0
    repsel9 = np.zeros((72, 72), np.float32)
    csel = np.zeros((72, 128), np.float32)
    for g in range(3):
        for h in range(HEADS):
            for j in range(9):
                repsel9[32 * g + h, 8 * j + h] = 1.0
        for p in range(128):
            csel[32 * g + p // 16, p] = 1.0
    ident = np.eye(128, dtype=np.float32)
    onesk = np.ones((128, 1), np.float32)
    out = dict(selqk=selqk, gsel0=gsel0, gsel1=gsel1, wsel0=wsel0, wsel1=wsel1,
               repsel9=repsel9, csel=csel, ident=ident, onesk=onesk)
    return {k: v.astype(bf) for k, v in out.items()}


@functools.lru_cache(maxsize=1)
def _build_module():
    nc = bacc.Bacc("TRN2", target_bir_lowering=False, debug=False)
    ins = {}

    def din(name, shape, dt):
        ins[name] = nc.dram_tensor(name, shape, dt, kind="ExternalInput").ap()

    din("xp", [NCH, 128, A], BF16)
    din("wqkv", [NCH, 128, 2304], BF16)
    din("wproj", [NCH, 128, 768], BF16)
    din("wfc", [NCH, 128, 256], BF16)
    din("bqkv", [128, 18], F32)
    din("bproj", [128, NCH], F32)
    din("bfc", [128, 2], F32)
    din("selqk", [NCH, 128, 248], BF16)
    din("gsel0", [128, 72], BF16)
    din("gsel1", [72, 72], BF16)
    din("wsel0", [128, 9, 72], BF16)
    din("wsel1", [72, 9, 72], BF16)
    din("repsel9", [72, 72], BF16)
    din("csel", [72, 128], BF16)
    din("ident", [128, 128], BF16)
    din("onesk", [128, 1], BF16)
    outs = {"y": nc.dram_tensor("y", [2, 128, 32, 32], F32,
                                kind="ExternalOutput").ap()}

    from contextlib import ExitStack
    with tile.TileContext(nc) as tc:
        with ExitStack() as ctx:
            with nc.allow_low_precision(reason="bf16 kernel by design"):
                emit_kernel(ctx, tc, ins, outs)
    nc.compile()
    return nc


def kernel(x, w_qkv, b_qkv, w_proj, b_proj, g1, beta1, g2, beta2, w_fc, b_fc,
           _run_kwargs=None):
    bf = ml_dtypes.bfloat16
    x = np.asarray(x, np.float32)
    B = x.shape[0]
    assert x.shape == (8, C, 32, 32)

    w_qkv = np.asarray(w_qkv, np.float32)
    b_qkv = np.asarray(b_qkv, np.float32)
    w_fc = np.asarray(w_fc, np.float32)
    b_fc = np.asarray(b_fc, np.float32)
    g1 = np.asarray(g1, np.float32)
    beta1 = np.asarray(beta1, np.float32)
    g2 = np.asarray(g2, np.float32)
    beta2 = np.asarray(beta2, np.float32)
    wq = g1[:, None] * w_qkv
    bq = beta1 @ w_qkv + b_qkv
    # 16-interleave v out-channels (head = p//16 within every 128-chunk) and
    # permute proj rows to match
    old_of_new = np.array([96 * ((n % 128) // 16) + 16 * (n // 128)
                           + (n % 16) for n in range(C)])
    wq[:, 1536:] = wq[:, 1536 + old_of_new]
    bq[1536:] = bq[1536 + old_of_new]
    w_proj = np.asarray(w_proj, np.float32)[old_of_new, :]
    wf = g2[:, None] * w_fc
    bfc2 = beta2 @ w_fc + b_fc

    sels = _build_sels()
    shared = dict(
        wqkv=np.ascontiguousarray(wq.reshape(NCH, 128, 2304)).astype(bf),
        wproj=np.ascontiguousarray(
            np.asarray(w_proj, np.float32).reshape(NCH, 128, 768)).astype(bf),
        wfc=np.ascontiguousarray(wf.reshape(NCH, 128, 256)).astype(bf),
        bqkv=np.ascontiguousarray(bq.reshape(18, 128).T),
        bproj=np.ascontiguousarray(
            np.asarray(b_proj, np.float32).reshape(NCH, 128).T),
        bfc=np.ascontiguousarray(bfc2.reshape(2, 128).T),
        **sels,
    )
    in_maps = []
    for b in range(B):
        xpad = np.pad(x[b], ((0, 0), (1, 1), (1, 1)), mode="edge")
        xp = np.ascontiguousarray(xpad.reshape(NCH, 128, A)).astype(bf)
        in_maps.append(dict(xp=xp, **shared))

    nc = _build_module()
    res = run_bass_kernel_spmd(nc, in_maps, core_ids=list(range(8)),
                               **(_run_kwargs or {}))
    outs = []
    for b in range(B):
        y = np.asarray(res.results[b]["y"], np.float32)  # [2,128,32,32]
        outs.append(y.reshape(256, 32, 32))
    out = np.stack(outs).astype(np.float32)
    if _run_kwargs is not None:
        kernel.last_result = res
    return out


# revision 5
# speedup vs baseline: 1.0785x; 1.0599x over previous
"""Trainium2 Bass kernel for nn_AttnBlock (sparse 3x3-window attention block).

Structure (per core, one batch image):
  - LN1/qkv computed once per padded pixel (34x34 grid); g1/beta1 folded into
    w_qkv/b_qkv and g2/beta2 into w_fc/b_fc on the host, so LN applies are two
    plain DVE ops.
  - Scores deduplicated into 25 displacement maps E_e = q . shift_e(k)
    (DVE/Pool elementwise product + PE per-head column reduction).
  - Softmax denominators via box-sum selection matmul (gsel, scaled x9 to fold
    the window mean), W = sum_ki (wsel_ki @ F)(shift) * R-broadcast.
  - o_mean = sum_kj shift(v * W-broadcast); broadcasts via PE selection
    matmuls, PSUM->SBUF bf16 copies on Act/Pool so big DVE products run in
    2x (16-bit) mode, full width.
  - Box filter (residual mean) runs on the Pool engine under the qkv phase.

Sharding: data-parallel over batch B=8 -> one batch per NeuronCore.
"""

import functools
import numpy as np
import ml_dtypes

import concourse.bass as bass
import concourse.mybir as mybir
import concourse.tile as tile
from concourse import bacc
from concourse.bass_utils import run_bass_kernel_spmd

F32 = mybir.dt.float32
BF16 = mybir.dt.bfloat16
AF = mybir.ActivationFunctionType
ALU = mybir.AluOpType

C = 768
NCH = 6          # channel chunks of 128
G = 34           # padded grid side
A = G * G        # 1156 padded pixels
AW = 1160        # padded-pixel width with 4 pad cols
NW = 1088        # window-grid width = 32*34 (rows 0..31, cols 0..33)
KW = 1300        # k map width with +-70 margins (content at 70)
HEADS = 8
HD = 96
SCALE = HD ** -0.5
EPS = 1e-5

SEG_A = [(0, 386), (386, 386), (772, 384)]   # PSUM segs over the a-grid
SEG_N = [(0, 363), (363, 363), (726, 362)]   # PSUM segs over the n-grid

E_LIST = [(er, ec) for er in range(-2, 3) for ec in range(-2, 3)]  # 25
KI_LIST = [(r, c) for r in range(3) for c in range(3)]             # 9

POOL_EI = {4, 9, 14, 19, 24}  # displacement products offloaded to Pool


def emit_kernel(ctx, tc, ins, outs):
    nc = tc.nc
    xp_d = ins["xp"]          # [6,128,1156] bf16
    wq_d = ins["wqkv"]        # [6,128,2304] bf16 (g1-folded)
    wp_d = ins["wproj"]       # [6,128,768] bf16
    wf_d = ins["wfc"]         # [6,128,256] bf16 (g2-folded)
    bqkv_d = ins["bqkv"]      # [128,18] f32 (beta1-folded)
    bproj_d = ins["bproj"]    # [128,6] f32
    bfc_d = ins["bfc"]        # [128,2] f32 (beta2-folded)
    selqk_d = ins["selqk"]    # [6,128,248] bf16
    gsel0_d = ins["gsel0"]    # [128,72] bf16 (x9)
    gsel1_d = ins["gsel1"]    # [72,72] bf16 (x9)
    wsel0_d = ins["wsel0"]    # [128,9,72] bf16
    wsel1_d = ins["wsel1"]    # [72,9,72] bf16
    repsel9_d = ins["repsel9"]  # [72,72] bf16
    csel_d = ins["csel"]      # [72,128] bf16
    ident_d = ins["ident"]    # [128,128] bf16
    onesk_d = ins["onesk"]    # [128,1] bf16
    y_d = outs["y"]           # [2,128,32,32] f32

    consts = ctx.enter_context(tc.tile_pool(name="consts", bufs=1))
    big = ctx.enter_context(tc.tile_pool(name="big", bufs=1))
    prodp = ctx.enter_context(tc.tile_pool(name="prodp", bufs=5))
    brp = ctx.enter_context(tc.tile_pool(name="brp", bufs=2))
    small = ctx.enter_context(tc.tile_pool(name="small", bufs=1))
    psA = ctx.enter_context(tc.tile_pool(name="psA", bufs=1, space="PSUM"))
    psB = ctx.enter_context(tc.tile_pool(name="psB", bufs=1, space="PSUM"))
    psC = ctx.enter_context(tc.tile_pool(name="psC", bufs=2, space="PSUM"))
    drp = ctx.enter_context(tc.tile_pool(name="drp", bufs=2, space="DRAM"))

    def psa():
        return psA.tile([128, 3, 512], F32, tag="a", name="psa_t")

    def psb():
        return psB.tile([128, 3, 512], F32, tag="b", name="psb_t")

    # two-op PSUM->SBUF copy helpers (banks 0-1 fused, bank 2), SEG-uniform
    def copy2(eng, dst01, dst2, ps, segs, m=128, func=AF.Copy, bias=None,
              scale=1.0):
        w01, w2 = segs[0][1], segs[2][1]
        if eng == "act":
            kw = {} if bias is None else {"bias": bias}
            nc.scalar.activation(dst01[0:m], ps[0:m, 0:2, 0:w01], func,
                                 scale=scale, **kw)
            nc.scalar.activation(dst2[0:m], ps[0:m, 2, 0:w2], func,
                                 scale=scale, **kw)
        elif eng == "pool":
            nc.gpsimd.tensor_copy(dst01[0:m], ps[0:m, 0:2, 0:w01])
            nc.gpsimd.tensor_copy(dst2[0:m], ps[0:m, 2, 0:w2])
        else:
            nc.vector.tensor_copy(dst01[0:m], ps[0:m, 0:2, 0:w01])
            nc.vector.tensor_copy(dst2[0:m], ps[0:m, 2, 0:w2])

    # ---- load constants ----
    def load(pool, name, shape, dt, src, tag=None):
        t = pool.tile(shape, dt, tag=tag or name, name=name)
        nc.sync.dma_start(out=t, in_=src)
        return t

    xpb = big.tile([128, NCH, AW], BF16, tag="bigA", name="xpb")
    onesk_t = load(consts, "onesk", [128, 1], BF16, onesk_d)
    for c in range(NCH):
        nc.sync.dma_start(out=xpb[:, c, 0:A], in_=xp_d[c])
    nc.vector.memset(xpb[:, :, A:AW], 0.0)
    wq_t = consts.tile([128, NCH, 2304], BF16, tag="wq", name="wq_t")
    wp_t = consts.tile([128, NCH, 768], BF16, tag="wp", name="wp_t")
    wf_t = consts.tile([128, NCH, 256], BF16, tag="wf", name="wf_t")
    selqk_t = consts.tile([128, NCH, 248], BF16, tag="selqk", name="selqk_t")
    for c in range(NCH):
        nc.sync.dma_start(out=wq_t[:, c, :], in_=wq_d[c])
        nc.sync.dma_start(out=wp_t[:, c, :], in_=wp_d[c])
        nc.sync.dma_start(out=wf_t[:, c, :], in_=wf_d[c])
        nc.sync.dma_start(out=selqk_t[:, c, :], in_=selqk_d[c])
    gsel0_t = load(consts, "gsel0", [128, 72], BF16, gsel0_d)
    gsel1_t = load(consts, "gsel1", [72, 72], BF16, gsel1_d)
    wsel0_t = load(consts, "wsel0", [128, 9, 72], BF16, wsel0_d)
    wsel1_t = load(consts, "wsel1", [72, 9, 72], BF16, wsel1_d)
    repsel9_t = load(consts, "repsel9", [72, 72], BF16, repsel9_d)
    csel_t = load(consts, "csel", [72, 128], BF16, csel_d)
    ident_t = load(consts, "ident", [128, 128], BF16, ident_d)
    bqkv_t = load(small, "bqkv", [128, 18], F32, bqkv_d)
    bproj_t = load(small, "bproj", [128, NCH], F32, bproj_d)
    bfc_t = load(small, "bfc", [128, 2], F32, bfc_d)

    # =================== box filter on Pool (residual t_mean, x9) ============
    t9 = big.tile([128, NCH, NW], BF16, tag="t9", name="t9")
    for c in range(NCH):
        tr = brp.tile([128, 1158], BF16, tag="wr", name="tr")
        nc.gpsimd.tensor_tensor(tr, xpb[:, c, 0:1158], xpb[:, c, 1:1159],
                                ALU.add)
        nc.gpsimd.tensor_tensor(tr, tr, xpb[:, c, 2:1160], ALU.add)
        nc.gpsimd.tensor_tensor(t9[:, c, :], tr[:, 0:NW], tr[:, 34:34 + NW],
                                ALU.add)
        nc.gpsimd.tensor_tensor(t9[:, c, :], t9[:, c, :], tr[:, 68:68 + NW],
                                ALU.add)

    # =================== LayerNorm 1 (stats over channels via PE) ============
    stat1 = psa()   # sum x   [1, a]
    stat2 = psb()   # sum x^2 [1, a]
    for c in range(NCH):
        sqx = prodp.tile([128, A], BF16, tag="pr", name="sqx")
        nc.vector.tensor_tensor(sqx, xpb[:, c, 0:A], xpb[:, c, 0:A], ALU.mult)
        for s, (off, w) in enumerate(SEG_A):
            nc.tensor.matmul(stat1[0:1, s, 0:w], onesk_t,
                             xpb[:, c, off:off + w],
                             start=(c == 0), stop=(c == NCH - 1),
                             skip_group_check=True)
            nc.tensor.matmul(stat2[0:1, s, 0:w], onesk_t,
                             sqx[:, off:off + w],
                             start=(c == 0), stop=(c == NCH - 1),
                             skip_group_check=True)

    def ln_smalls(stat1, stat2, width, segs, tagpfx):
        """From PSUM sums -> rstd (bf16) and -mu*rstd (bf16), [1, width]."""
        ta = small.tile([1, width], F32, tag="lnta", name=tagpfx + "ta")
        xs = small.tile([1, width], F32, tag="lnxs", name=tagpfx + "xs")
        sq = small.tile([1, width], F32, tag="lnta", name=tagpfx + "sv")
        rstd = small.tile([1, width], BF16, tag="lnrs", name=tagpfx + "rs")
        nmur = small.tile([1, width], BF16, tag="lnnm", name=tagpfx + "nm")
        eps_t = small.tile([1, 1], F32, tag="lnep", name=tagpfx + "ep")
        nc.vector.memset(eps_t, EPS)
        for s, (off, w) in enumerate(segs):
            nc.scalar.activation(ta[:, off:off + w], stat1[0:1, s, 0:w],
                                 AF.Square)
            # xs = ta/768 - stat2   (= -768*var)
            nc.vector.scalar_tensor_tensor(xs[:, off:off + w],
                                           ta[:, off:off + w], 1.0 / C,
                                           stat2[0:1, s, 0:w],
                                           ALU.mult, ALU.subtract)
        # sq = sqrt(xs * (-1/768) + eps) = sqrt(var + eps)
        for s, (off, w) in enumerate(segs):
            nc.scalar.activation(sq[:, off:off + w], xs[:, off:off + w],
                                 AF.Sqrt, bias=eps_t, scale=-1.0 / C)
            nc.vector.reciprocal(rstd[:, off:off + w], sq[:, off:off + w])
        # nmur = (stat1 * -1/768) * rstd = -mu * rstd  (stat1 still live: the
        # next user of its PSUM tile is WAR-fenced behind this read)
        for s, (off, w) in enumerate(segs):
            nc.vector.scalar_tensor_tensor(nmur[:, off:off + w],
                                           stat1[0:1, s, 0:w], -1.0 / C,
                                           rstd[:, off:off + w],
                                           ALU.mult, ALU.mult)
        return rstd, nmur

    rstd1, nmur1 = ln_smalls(stat1, stat2, A, SEG_A, "l1")

    # broadcast rstd / nmur to 128 partitions via partition-step-0 DMA
    rrep1 = small.tile([128, A], BF16, tag="lnrr", name="rrep1")
    nrep1 = small.tile([128, A], BF16, tag="lnnr", name="nrep1")
    rscr1 = drp.tile([1, A], BF16, tag="scr", name="rscr1")
    nscr1 = drp.tile([1, A], BF16, tag="scr", name="nscr1")
    nc.sync.dma_start(out=rscr1, in_=rstd1)
    nc.sync.dma_start(out=nscr1, in_=nmur1)
    nc.sync.dma_start(out=rrep1, in_=rscr1.to_broadcast([128, A]))
    nc.sync.dma_start(out=nrep1, in_=nscr1.to_broadcast([128, A]))

    ln_b = big.tile([128, NCH, A], BF16, tag="ln", name="ln_b")
    for s_, (off, w) in enumerate(SEG_A):
        for c in range(NCH):
            t1 = prodp.tile([128, A], BF16, tag="pr", name="t1")
            nc.vector.tensor_tensor(t1[:, 0:w], xpb[:, c, off:off + w],
                                    rrep1[:, off:off + w], ALU.mult)
            nc.vector.tensor_tensor(ln_b[:, c, off:off + w], t1[:, 0:w],
                                    nrep1[:, off:off + w], ALU.add)

    # =================== qkv projection ======================================
    qp = big.tile([128, NCH, AW], BF16, tag="qo", name="qp")
    kp = big.tile([128, NCH, KW], BF16, tag="kp", name="kp")
    vp = big.tile([128, NCH, AW], BF16, tag="vp", name="vp")
    nc.vector.memset(qp[:, :, A:AW], 0.0)
    nc.vector.memset(kp[:, :, 0:70], 0.0)
    nc.vector.memset(kp[:, :, 70 + A:KW], 0.0)

    for g in range(18):
        ps = psa() if g % 2 == 0 else psb()
        for s, (off, w) in enumerate(SEG_A):
            for c in range(NCH):
                nc.tensor.matmul(ps[:, s, 0:w],
                                 wq_t[:, c, 128 * g:128 * (g + 1)],
                                 ln_b[:, c, off:off + w],
                                 start=(c == 0), stop=(c == NCH - 1))
        if g < 6:
            dst = qp[:, g, :]
        elif g < 12:
            dst = kp[:, g - 6, 70:]
        else:
            dst = vp[:, g - 12, :]
        copy2("act", dst[:, 0:772], dst[:, 772:1156], ps, SEG_A,
              func=AF.Identity, bias=bqkv_t[:, g:g + 1])
    # vp pad must be zero for shifted o-products
    nc.vector.memset(vp[:, :, A:AW], 0.0)

    # =================== scores: 25 displacement maps ========================
    E0 = psa()                      # [(16e x 8h), a]
    E1 = psb()                      # [(9e x 8h), a]
    for c in range(NCH):
        for ei, (er, ec) in enumerate(E_LIST):
            grp, j = (0, ei) if ei < 16 else (1, ei - 16)
            koff = 70 + 34 * er + ec
            Eg = E0 if grp == 0 else E1
            m = 128 if grp == 0 else 72
            prod = prodp.tile([128, A], BF16, tag="pr", name="prod")
            if ei in POOL_EI:
                nc.gpsimd.tensor_tensor(prod, qp[:, c, 0:A],
                                        kp[:, c, koff:koff + A], ALU.mult)
            else:
                nc.vector.tensor_tensor(prod, qp[:, c, 0:A],
                                        kp[:, c, koff:koff + A], ALU.mult)
            lhs = selqk_t[:, c, 120 - 8 * j:120 - 8 * j + m]
            first = (c == 0 and j == 0)
            last = (c == NCH - 1 and j == (15 if grp == 0 else 8))
            for s, (off, w) in enumerate(SEG_A):
                nc.tensor.matmul(Eg[0:m, s, 0:w], lhs, prod[:, off:off + w],
                                 start=first, stop=last, skip_group_check=True)

    F0 = big.tile([128, AW], BF16, tag="F0", name="F0")
    F1 = big.tile([72, AW], BF16, tag="F1", name="F1")
    nc.vector.memset(F0[:, A:AW], 0.0)
    nc.vector.memset(F1[:, A:AW], 0.0)
    for Ft, Eg, m in ((F0, E0, 128), (F1, E1, 72)):
        copy2("act", Ft[:, 0:772], Ft[:, 772:1156], Eg, SEG_A, m=m,
              func=AF.Exp, scale=SCALE)

    # =================== denominators: G (x9) -> R = 1/(9G) ==================
    Gp = psa()
    for s, (off, w) in enumerate(SEG_A):
        nc.tensor.matmul(Gp[0:72, s, 0:w], gsel0_t, F0[:, off:off + w],
                         start=True, stop=False)
        nc.tensor.matmul(Gp[0:72, s, 0:w], gsel1_t, F1[0:72, off:off + w],
                         start=False, stop=True, skip_group_check=True)
    R9all = consts.tile([72, 3, AW], BF16, tag="wq", name="R9all")
    R = big.tile([72, AW], BF16, tag="R", name="R")
    nc.vector.memset(R[:, A:AW], 0.0)
    nc.vector.memset(R9all[:, :, A:AW], 0.0)
    nc.vector.reciprocal(R[:, 0:772], Gp[0:72, 0:2, 0:386])
    nc.vector.reciprocal(R[:, 772:1156], Gp[0:72, 2, 0:384])
    R9s = [R9all[:, t, :] for t in range(3)]
    for ki in range(9):
        t, g = divmod(ki, 3)
        nc.sync.dma_start(out=R9s[t][32 * g:32 * g + 8, :],
                          in_=R[8 * ki:8 * ki + 8, :])

    # =================== W = sum_ki (wsel_ki @ F)(shift ski) * R_rep =========
    W_acc = big.tile([72, NW], BF16, tag="Wa", name="W_acc")
    for ki, (kir, kic) in enumerate(KI_LIST):
        t, gg = divmod(ki, 3)
        ski = 34 * kir + kic
        Bp = psa()   # B_ki = wsel @ F(shift)
        Rp = psb()   # rrep' = R row (shift) broadcast to 72 rows
        for s, (off, w) in enumerate(SEG_N):
            nc.tensor.matmul(Bp[0:72, s, 0:w], wsel0_t[:, ki, :],
                             F0[:, ski + off:ski + off + w],
                             start=True, stop=False)
            nc.tensor.matmul(Bp[0:72, s, 0:w], wsel1_t[0:72, ki, :],
                             F1[0:72, ski + off:ski + off + w],
                             start=False, stop=True, skip_group_check=True)
            nc.tensor.matmul(Rp[0:72, s, 0:w],
                             repsel9_t[32 * gg:32 * gg + 8, 0:72],
                             R9s[t][32 * gg:32 * gg + 8,
                                    ski + off:ski + off + w],
                             start=True, stop=True)
        B_b = brp.tile([72, NW], BF16, tag="bb", name="B_b")
        copy2("act", B_b[:, 0:726], B_b[:, 726:1088], Bp, SEG_N, m=72)
        # (B * 1.0) * rrep'  -- STT reads rrep' straight from PSUM
        dst = W_acc if ki == 0 else brp.tile([72, NW], BF16, tag="wt",
                                             name="W_tmp")
        nc.vector.scalar_tensor_tensor(dst[:, 0:726], B_b[:, 0:726], 1.0,
                                       Rp[0:72, 0:2, 0:363],
                                       ALU.mult, ALU.mult)
        nc.vector.scalar_tensor_tensor(dst[:, 726:1088], B_b[:, 726:1088],
                                       1.0, Rp[0:72, 2, 0:362],
                                       ALU.mult, ALU.mult)
        if ki > 0:
            nc.vector.tensor_tensor(W_acc, W_acc, dst, ALU.add)

    W9all = big.tile([72, 3, NW], BF16, tag="W9", name="W9all")
    W9s = [W9all[:, t, :] for t in range(3)]
    for kj in range(9):
        t, g = divmod(kj, 3)
        nc.sync.dma_start(out=W9s[t][32 * g:32 * g + 8, :],
                          in_=W_acc[8 * kj:8 * kj + 8, :])

    # =================== o_mean accumulation ================================
    # v channels are 16-interleaved (head = p//16 in every chunk), so one
    # W-broadcast per kj serves all 6 chunks; materialize all 9 upfront.
    wr_all = big.tile([128, 9, NW], BF16, tag="kp", name="wr_all")
    for kj in range(9):
        t, gg = divmod(kj, 3)
        wps = psa() if kj % 2 == 0 else psb()
        for s_, (off, w) in enumerate(SEG_N):
            nc.tensor.matmul(wps[:, s_, 0:w],
                             csel_t[32 * gg:32 * gg + 8, :],
                             W9s[t][32 * gg:32 * gg + 8, off:off + w],
                             start=True, stop=True)
        copy2("act", wr_all[:, kj, 0:726], wr_all[:, kj, 726:1088], wps, SEG_N)
    o_b = big.tile([128, NCH, NW], BF16, tag="qo", name="o_b")
    for c in range(NCH):
        oacc = psa() if c % 2 == 0 else psb()
        # kj 7,8 on Pool, issued first, consumed at the end of the chain
        pprods = []
        for kj in (7, 8):
            kjr, kjc = KI_LIST[kj]
            skj = 34 * kjr + kjc
            pp = brp.tile([128, NW], BF16, tag="pp", name="prodp_" + str(kj))
            nc.gpsimd.tensor_tensor(pp, vp[:, c, skj:skj + NW],
                                    wr_all[:, kj, :], ALU.mult)
            pprods.append(pp)
        for kj in range(7):
            kjr, kjc = KI_LIST[kj]
            skj = 34 * kjr + kjc
            prod = prodp.tile([128, NW], BF16, tag="pr", name="prodo")
            nc.vector.tensor_tensor(prod, vp[:, c, skj:skj + NW],
                                    wr_all[:, kj, :], ALU.mult)
            for s_, (off, w) in enumerate(SEG_N):
                nc.tensor.matmul(oacc[:, s_, 0:w], ident_t,
                                 prod[:, off:off + w],
                                 start=(kj == 0), stop=False,
                                 skip_group_check=True)
        for i2, kj in enumerate((7, 8)):
            for s_, (off, w) in enumerate(SEG_N):
                nc.tensor.matmul(oacc[:, s_, 0:w], ident_t,
                                 pprods[i2][:, off:off + w],
                                 start=False, stop=(kj == 8),
                                 skip_group_check=True)
        copy2("act", o_b[:, c, 0:726], o_b[:, c, 726:1088], oacc, SEG_N)

    # =================== proj + residual -> u ================================
    u_b = big.tile([128, NCH, NW], BF16, tag="ub", name="u_b")
    sq2_all = big.tile([128, NCH, NW], BF16, tag="vp", name="sq2_all")
    for g in range(NCH):
        ps = psa() if g % 2 == 0 else psb()
        for s, (off, w) in enumerate(SEG_N):
            for c in range(NCH):
                nc.tensor.matmul(ps[:, s, 0:w],
                                 wp_t[:, c, 128 * g:128 * (g + 1)],
                                 o_b[:, c, off:off + w],
                                 start=(c == 0), stop=(c == NCH - 1))
        # u = t9/9 + r, then + bproj
        nc.vector.scalar_tensor_tensor(u_b[:, g, 0:726], t9[:, g, 0:726],
                                       1.0 / 9.0, ps[:, 0:2, 0:363],
                                       ALU.mult, ALU.add)
        nc.vector.scalar_tensor_tensor(u_b[:, g, 726:1088], t9[:, g, 726:1088],
                                       1.0 / 9.0, ps[:, 2, 0:362],
                                       ALU.mult, ALU.add)
        nc.vector.tensor_scalar_add(u_b[:, g, :], u_b[:, g, :],
                                    bproj_t[:, g:g + 1])
        nc.scalar.activation(sq2_all[:, g, :], u_b[:, g, :], AF.Square)

    # =================== LayerNorm 2 ========================================
    stat1b = psa()
    stat2b = psb()
    for c in range(NCH):
        for s, (off, w) in enumerate(SEG_N):
            nc.tensor.matmul(stat1b[0:1, s, 0:w], onesk_t,
                             u_b[:, c, off:off + w],
                             start=(c == 0), stop=(c == NCH - 1),
                             skip_group_check=True)
            nc.tensor.matmul(stat2b[0:1, s, 0:w], onesk_t,
                             sq2_all[:, c, off:off + w],
                             start=(c == 0), stop=(c == NCH - 1),
                             skip_group_check=True)
    rstd2, nmur2 = ln_smalls(stat1b, stat2b, NW, SEG_N, "l2")
    rrep2 = small.tile([128, NW], BF16, tag="lnrr", name="rrep2")
    nrep2 = small.tile([128, NW], BF16, tag="lnnr", name="nrep2")
    rscr2 = drp.tile([1, NW], BF16, tag="scr", name="rscr2")
    nscr2 = drp.tile([1, NW], BF16, tag="scr", name="nscr2")
    nc.sync.dma_start(out=rscr2, in_=rstd2)
    nc.sync.dma_start(out=nscr2, in_=nmur2)
    nc.sync.dma_start(out=rrep2, in_=rscr2.to_broadcast([128, NW]))
    nc.sync.dma_start(out=nrep2, in_=nscr2.to_broadcast([128, NW]))
    ln2_b = big.tile([128, NCH, NW], BF16, tag="ln", name="ln2_b")
    for s_, (off, w) in enumerate(SEG_N):
        for c in range(NCH):
            t1 = prodp.tile([128, NW], BF16, tag="pr", name="t1b")
            nc.vector.tensor_tensor(t1[:, 0:w], u_b[:, c, off:off + w],
                                    rrep2[:, off:off + w], ALU.mult)
            nc.vector.tensor_tensor(ln2_b[:, c, off:off + w], t1[:, 0:w],
                                    nrep2[:, off:off + w], ALU.add)

    # =================== fc + relu + output =================================
    y_t = big.tile([128, 2, NW], F32, tag="kp", name="y_t")
    for g in range(2):
        ps = psa() if g % 2 == 0 else psb()
        for s, (off, w) in enumerate(SEG_N):
            for c in range(NCH):
                nc.tensor.matmul(ps[:, s, 0:w],
                                 wf_t[:, c, 128 * g:128 * (g + 1)],
                                 ln2_b[:, c, off:off + w],
                                 start=(c == 0), stop=(c == NCH - 1))
        copy2("act", y_t[:, g, 0:726], y_t[:, g, 726:1088], ps, SEG_N,
              func=AF.Relu, bias=bfc_t[:, g:g + 1])
        src = y_t[:, g, :].rearrange("p (r c) -> p r c", c=34)[:, :, 0:32]
        nc.sync.dma_start(out=y_d[g], in_=src)


# ============================ host-side wrapper =============================

def _build_sels():
    bf = ml_dtypes.bfloat16
    selqk = np.zeros((NCH, 128, 248), np.float32)
    for c in range(NCH):
        for r in range(128):
            h = (128 * c + r) // HD
            selqk[c, r, 120 + h] = 1.0
    gsel0 = np.zeros((128, 72), np.float32)
    gsel1 = np.zeros((72, 72), np.float32)
    for ki, (kir, kic) in enumerate(KI_LIST):
        for j, (er, ec) in enumerate(E_LIST):
            if -kir <= er <= 2 - kir and -kic <= ec <= 2 - kic:
                for h in range(HEADS):
                    if j < 16:
                        gsel0[8 * j + h, 8 * ki + h] = 9.0
                    else:
                        gsel1[8 * (j - 16) + h, 8 * ki + h] = 9.0
    wsel0 = np.zeros((128, 9, 72), np.float32)
    wsel1 = np.zeros((72, 9, 72), np.float32)
    for ki, (kir, kic) in enumerate(KI_LIST):
        for j, (er, ec) in enumerate(E_LIST):
            kjr, kjc = er + kir, ec + kic
            if 0 <= kjr <= 2 and 0 <= kjc <= 2:
                kj = 3 * kjr + kjc
                for h in range(HEADS):
                    if j < 16:
                        wsel0[8 * j + h, ki, 8 * kj + h] = 1.0
                    else:
                        wsel1[8 * (j - 16) + h, ki, 8 * kj + h] = 1.0
    repsel9 = np.zeros((72, 72), np.float32)
    csel = np.zeros((72, 128), np.float32)
    for g in range(3):
        for h in range(HEADS):
            for j in range(9):
                repsel9[32 * g + h, 8 * j + h] = 1.0
        for p in range(128):
            csel[32 * g + p // 16, p] = 1.0
    ident = np.eye(128, dtype=np.float32)
    onesk = np.ones((128, 1), np.float32)
    out = dict(selqk=selqk, gsel0=gsel0, gsel1=gsel1, wsel0=wsel0, wsel1=wsel1,
               repsel9=repsel9, csel=csel, ident=ident, onesk=onesk)
    return {k: v.astype(bf) for k, v in out.items()}


@functools.lru_cache(maxsize=1)
def _build_module():
    nc = bacc.Bacc("TRN2", target_bir_lowering=False, debug=False)
    ins = {}

    def din(name, shape, dt):
        ins[name] = nc.dram_tensor(name, shape, dt, kind="ExternalInput").ap()

    din("xp", [NCH, 128, A], BF16)
    din("wqkv", [NCH, 128, 2304], BF16)
    din("wproj", [NCH, 128, 768], BF16)
    din("wfc", [NCH, 128, 256], BF16)
    din("bqkv", [128, 18], F32)
    din("bproj", [128, NCH], F32)
    din("bfc", [128, 2], F32)
    din("selqk", [NCH, 128, 248], BF16)
    din("gsel0", [128, 72], BF16)
    din("gsel1", [72, 72], BF16)
    din("wsel0", [128, 9, 72], BF16)
    din("wsel1", [72, 9, 72], BF16)
    din("repsel9", [72, 72], BF16)
    din("csel", [72, 128], BF16)
    din("ident", [128, 128], BF16)
    din("onesk", [128, 1], BF16)
    outs = {"y": nc.dram_tensor("y", [2, 128, 32, 32], F32,
                                kind="ExternalOutput").ap()}

    from contextlib import ExitStack
    with tile.TileContext(nc) as tc:
        with ExitStack() as ctx:
            with nc.allow_low_precision(reason="bf16 kernel by design"):
                emit_kernel(ctx, tc, ins, outs)
    nc.compile()
    return nc


def kernel(x, w_qkv, b_qkv, w_proj, b_proj, g1, beta1, g2, beta2, w_fc, b_fc,
           _run_kwargs=None):
    bf = ml_dtypes.bfloat16
    x = np.asarray(x, np.float32)
    B = x.shape[0]
    assert x.shape == (8, C, 32, 32)

    w_qkv = np.asarray(w_qkv, np.float32)
    b_qkv = np.asarray(b_qkv, np.float32)
    w_fc = np.asarray(w_fc, np.float32)
    b_fc = np.asarray(b_fc, np.float32)
    g1 = np.asarray(g1, np.float32)
    beta1 = np.asarray(beta1, np.float32)
    g2 = np.asarray(g2, np.float32)
    beta2 = np.asarray(beta2, np.float32)
    wq = g1[:, None] * w_qkv
    bq = beta1 @ w_qkv + b_qkv
    # 16-interleave v out-channels (head = p//16 within every 128-chunk) and
    # permute proj rows to match
    old_of_new = np.array([96 * ((n % 128) // 16) + 16 * (n // 128)
                           + (n % 16) for n in range(C)])
    wq[:, 1536:] = wq[:, 1536 + old_of_new]
    bq[1536:] = bq[1536 + old_of_new]
    w_proj = np.asarray(w_proj, np.float32)[old_of_new, :]
    wf = g2[:, None] * w_fc
    bfc2 = beta2 @ w_fc + b_fc

    sels = _build_sels()
    shared = dict(
        wqkv=np.ascontiguousarray(wq.reshape(NCH, 128, 2304)).astype(bf),
        wproj=np.ascontiguousarray(
            np.asarray(w_proj, np.float32).reshape(NCH, 128, 768)).astype(bf),
        wfc=np.ascontiguousarray(wf.reshape(NCH, 128, 256)).astype(bf),
        bqkv=np.ascontiguousarray(bq.reshape(18, 128).T),
        bproj=np.ascontiguousarray(
            np.asarray(b_proj, np.float32).reshape(NCH, 128).T),
        bfc=np.ascontiguousarray(bfc2.reshape(2, 128).T),
        **sels,
    )
    in_maps = []
    for b in range(B):
        xpad = np.pad(x[b], ((0, 0), (1, 1), (1, 1)), mode="edge")
        xp = np.ascontiguousarray(xpad.reshape(NCH, 128, A)).astype(bf)
        in_maps.append(dict(xp=xp, **shared))

    nc = _build_module()
    res = run_bass_kernel_spmd(nc, in_maps, core_ids=list(range(8)),
                               **(_run_kwargs or {}))
    outs = []
    for b in range(B):
        y = np.asarray(res.results[b]["y"], np.float32)  # [2,128,32,32]
        outs.append(y.reshape(256, 32, 32))
    out = np.stack(outs).astype(np.float32)
    if _run_kwargs is not None:
        kernel.last_result = res
    return out


# revision 6
# speedup vs baseline: 1.0886x; 1.0094x over previous
"""Trainium2 Bass kernel for nn_AttnBlock (sparse 3x3-window attention block).

Structure (per core, one batch image):
  - LN1/qkv computed once per padded pixel (34x34 grid); g1/beta1 folded into
    w_qkv/b_qkv and g2/beta2 into w_fc/b_fc on the host, so LN applies are two
    plain DVE ops.
  - Scores deduplicated into 25 displacement maps E_e = q . shift_e(k)
    (DVE/Pool elementwise product + PE per-head column reduction).
  - Softmax denominators via box-sum selection matmul (gsel, scaled x9 to fold
    the window mean), W = sum_ki (wsel_ki @ F)(shift) * R-broadcast.
  - o_mean = sum_kj shift(v * W-broadcast); broadcasts via PE selection
    matmuls, PSUM->SBUF bf16 copies on Act/Pool so big DVE products run in
    2x (16-bit) mode, full width.
  - Box filter (residual mean) runs on the Pool engine under the qkv phase.

Sharding: data-parallel over batch B=8 -> one batch per NeuronCore.
"""

import functools
import numpy as np
import ml_dtypes

import concourse.bass as bass
import concourse.mybir as mybir
import concourse.tile as tile
from concourse import bacc
from concourse.bass_utils import run_bass_kernel_spmd

F32 = mybir.dt.float32
BF16 = mybir.dt.bfloat16
AF = mybir.ActivationFunctionType
ALU = mybir.AluOpType

C = 768
NCH = 6          # channel chunks of 128
G = 34           # padded grid side
A = G * G        # 1156 padded pixels
AW = 1160        # padded-pixel width with 4 pad cols
NW = 1088        # window-grid width = 32*34 (rows 0..31, cols 0..33)
KW = 1300        # k map width with +-70 margins (content at 70)
HEADS = 8
HD = 96
SCALE = HD ** -0.5
EPS = 1e-5

SEG_A = [(0, 386), (386, 386), (772, 384)]   # PSUM segs over the a-grid
SEG_N = [(0, 363), (363, 363), (726, 362)]   # PSUM segs over the n-grid

E_LIST = [(er, ec) for er in range(-2, 3) for ec in range(-2, 3)]  # 25
KI_LIST = [(r, c) for r in range(3) for c in range(3)]             # 9

POOL_EI = {4, 9, 14, 19, 24}  # displacement products offloaded to Pool


def emit_kernel(ctx, tc, ins, outs):
    nc = tc.nc
    xp_d = ins["xp"]          # [6,128,1156] bf16
    wq_d = ins["wqkv"]        # [6,128,2304] bf16 (g1-folded)
    wp_d = ins["wproj"]       # [6,128,768] bf16
    wf_d = ins["wfc"]         # [6,128,256] bf16 (g2-folded)
    bqkv_d = ins["bqkv"]      # [128,18] f32 (beta1-folded)
    bproj_d = ins["bproj"]    # [128,6] f32
    bfc_d = ins["bfc"]        # [128,2] f32 (beta2-folded)
    selqk_d = ins["selqk"]    # [6,128,248] bf16
    gsel0_d = ins["gsel0"]    # [128,72] bf16 (x9)
    gsel1_d = ins["gsel1"]    # [72,72] bf16 (x9)
    wsel0_d = ins["wsel0"]    # [128,9,72] bf16
    wsel1_d = ins["wsel1"]    # [72,9,72] bf16
    repsel9_d = ins["repsel32"]  # [72,9,72] bf16
    csel_d = ins["csel32"]    # [72,9,128] bf16
    ident_d = ins["ident"]    # [128,128] bf16
    onesk_d = ins["onesk"]    # [128,1] bf16
    y_d = outs["y"]           # [2,128,32,32] f32

    consts = ctx.enter_context(tc.tile_pool(name="consts", bufs=1))
    big = ctx.enter_context(tc.tile_pool(name="big", bufs=1))
    prodp = ctx.enter_context(tc.tile_pool(name="prodp", bufs=6))
    brp = ctx.enter_context(tc.tile_pool(name="brp", bufs=2))
    small = ctx.enter_context(tc.tile_pool(name="small", bufs=1))
    psA = ctx.enter_context(tc.tile_pool(name="psA", bufs=1, space="PSUM"))
    psB = ctx.enter_context(tc.tile_pool(name="psB", bufs=1, space="PSUM"))
    psC = ctx.enter_context(tc.tile_pool(name="psC", bufs=2, space="PSUM"))
    drp = ctx.enter_context(tc.tile_pool(name="drp", bufs=2, space="DRAM"))

    def psa():
        return psA.tile([128, 3, 512], F32, tag="a", name="psa_t")

    def psb():
        return psB.tile([128, 3, 512], F32, tag="b", name="psb_t")

    # two-op PSUM->SBUF copy helpers (banks 0-1 fused, bank 2), SEG-uniform
    def copy2(eng, dst01, dst2, ps, segs, m=128, func=AF.Copy, bias=None,
              scale=1.0):
        w01, w2 = segs[0][1], segs[2][1]
        if eng == "act":
            kw = {} if bias is None else {"bias": bias}
            nc.scalar.activation(dst01[0:m], ps[0:m, 0:2, 0:w01], func,
                                 scale=scale, **kw)
            nc.scalar.activation(dst2[0:m], ps[0:m, 2, 0:w2], func,
                                 scale=scale, **kw)
        elif eng == "pool":
            nc.gpsimd.tensor_copy(dst01[0:m], ps[0:m, 0:2, 0:w01])
            nc.gpsimd.tensor_copy(dst2[0:m], ps[0:m, 2, 0:w2])
        else:
            nc.vector.tensor_copy(dst01[0:m], ps[0:m, 0:2, 0:w01])
            nc.vector.tensor_copy(dst2[0:m], ps[0:m, 2, 0:w2])

    # ---- load constants ----
    def load(pool, name, shape, dt, src, tag=None):
        t = pool.tile(shape, dt, tag=tag or name, name=name)
        nc.sync.dma_start(out=t, in_=src)
        return t

    xpb = big.tile([128, NCH, AW], BF16, tag="bigA", name="xpb")
    onesk_t = load(consts, "onesk", [128, 1], BF16, onesk_d)
    for c in range(NCH):
        nc.sync.dma_start(out=xpb[:, c, 0:A], in_=xp_d[c])
    nc.vector.memset(xpb[:, :, A:AW], 0.0)
    wq_t = consts.tile([128, NCH, 2304], BF16, tag="wq", name="wq_t")
    wp_t = consts.tile([128, NCH, 768], BF16, tag="wp", name="wp_t")
    wf_t = consts.tile([128, NCH, 256], BF16, tag="wf", name="wf_t")
    selqk_t = consts.tile([128, NCH, 248], BF16, tag="selqk", name="selqk_t")
    for c in range(NCH):
        nc.sync.dma_start(out=wq_t[:, c, :], in_=wq_d[c])
        nc.sync.dma_start(out=wp_t[:, c, :], in_=wp_d[c])
        nc.sync.dma_start(out=wf_t[:, c, :], in_=wf_d[c])
        nc.sync.dma_start(out=selqk_t[:, c, :], in_=selqk_d[c])
    gsel0_t = load(consts, "gsel0", [128, 72], BF16, gsel0_d)
    gsel1_t = load(consts, "gsel1", [72, 72], BF16, gsel1_d)
    wsel0_t = load(consts, "wsel0", [128, 9, 72], BF16, wsel0_d)
    wsel1_t = load(consts, "wsel1", [72, 9, 72], BF16, wsel1_d)
    repsel32_t = load(consts, "repsel32", [72, 9, 72], BF16,
                      repsel9_d)
    csel32_t = load(consts, "csel32", [72, 9, 128], BF16, csel_d)
    ident_t = load(consts, "ident", [128, 128], BF16, ident_d)
    bqkv_t = load(small, "bqkv", [128, 18], F32, bqkv_d)
    bproj_t = load(small, "bproj", [128, NCH], F32, bproj_d)
    bfc_t = load(small, "bfc", [128, 2], F32, bfc_d)

    # =================== box filter on Pool (residual t_mean, x9) ============
    t9 = big.tile([128, NCH, NW], BF16, tag="t9", name="t9")
    for c in range(NCH):
        tr = brp.tile([128, 1158], BF16, tag="wr", name="tr")
        nc.gpsimd.tensor_tensor(tr, xpb[:, c, 0:1158], xpb[:, c, 1:1159],
                                ALU.add)
        nc.gpsimd.tensor_tensor(tr, tr, xpb[:, c, 2:1160], ALU.add)
        nc.gpsimd.tensor_tensor(t9[:, c, :], tr[:, 0:NW], tr[:, 34:34 + NW],
                                ALU.add)
        nc.gpsimd.tensor_tensor(t9[:, c, :], t9[:, c, :], tr[:, 68:68 + NW],
                                ALU.add)

    # =================== LayerNorm 1 (stats over channels via PE) ============
    stat1 = psa()   # sum x   [1, a]
    stat2 = psb()   # sum x^2 [1, a]
    for c in range(NCH):
        sqx = prodp.tile([128, A], BF16, tag="pr", name="sqx")
        nc.vector.tensor_tensor(sqx, xpb[:, c, 0:A], xpb[:, c, 0:A], ALU.mult)
        for s, (off, w) in enumerate(SEG_A):
            nc.tensor.matmul(stat1[0:1, s, 0:w], onesk_t,
                             xpb[:, c, off:off + w],
                             start=(c == 0), stop=(c == NCH - 1),
                             skip_group_check=True)
            nc.tensor.matmul(stat2[0:1, s, 0:w], onesk_t,
                             sqx[:, off:off + w],
                             start=(c == 0), stop=(c == NCH - 1),
                             skip_group_check=True)

    def ln_smalls(stat1, stat2, width, segs, tagpfx):
        """From PSUM sums -> rstd (bf16) and -mu*rstd (bf16), [1, width]."""
        ta = small.tile([1, width], F32, tag="lnta", name=tagpfx + "ta")
        xs = small.tile([1, width], F32, tag="lnxs", name=tagpfx + "xs")
        sq = small.tile([1, width], F32, tag="lnta", name=tagpfx + "sv")
        rstd = small.tile([1, width], BF16, tag="lnrs", name=tagpfx + "rs")
        nmur = small.tile([1, width], BF16, tag="lnnm", name=tagpfx + "nm")
        eps_t = small.tile([1, 1], F32, tag="lnep", name=tagpfx + "ep")
        nc.vector.memset(eps_t, EPS)
        for s, (off, w) in enumerate(segs):
            nc.scalar.activation(ta[:, off:off + w], stat1[0:1, s, 0:w],
                                 AF.Square)
            # xs = ta/768 - stat2   (= -768*var)
            nc.vector.scalar_tensor_tensor(xs[:, off:off + w],
                                           ta[:, off:off + w], 1.0 / C,
                                           stat2[0:1, s, 0:w],
                                           ALU.mult, ALU.subtract)
        # sq = sqrt(xs * (-1/768) + eps) = sqrt(var + eps)
        for s, (off, w) in enumerate(segs):
            nc.scalar.activation(sq[:, off:off + w], xs[:, off:off + w],
                                 AF.Sqrt, bias=eps_t, scale=-1.0 / C)
            nc.vector.reciprocal(rstd[:, off:off + w], sq[:, off:off + w])
        # nmur = (stat1 * -1/768) * rstd = -mu * rstd  (stat1 still live: the
        # next user of its PSUM tile is WAR-fenced behind this read)
        for s, (off, w) in enumerate(segs):
            nc.vector.scalar_tensor_tensor(nmur[:, off:off + w],
                                           stat1[0:1, s, 0:w], -1.0 / C,
                                           rstd[:, off:off + w],
                                           ALU.mult, ALU.mult)
        return rstd, nmur

    rstd1, nmur1 = ln_smalls(stat1, stat2, A, SEG_A, "l1")

    # broadcast rstd / nmur to 128 partitions via partition-step-0 DMA
    rrep1 = small.tile([128, A], BF16, tag="lnrr", name="rrep1")
    nrep1 = small.tile([128, A], BF16, tag="lnnr", name="nrep1")
    rscr1 = drp.tile([1, A], BF16, tag="scr", name="rscr1")
    nscr1 = drp.tile([1, A], BF16, tag="scr", name="nscr1")
    nc.sync.dma_start(out=rscr1, in_=rstd1)
    nc.sync.dma_start(out=nscr1, in_=nmur1)
    nc.sync.dma_start(out=rrep1, in_=rscr1.to_broadcast([128, A]))
    nc.sync.dma_start(out=nrep1, in_=nscr1.to_broadcast([128, A]))

    ln_b = big.tile([128, NCH, A], BF16, tag="ln", name="ln_b")
    for s_, (off, w) in enumerate(SEG_A):
        for c in range(NCH):
            t1 = prodp.tile([128, A], BF16, tag="pr", name="t1")
            nc.vector.tensor_tensor(t1[:, 0:w], xpb[:, c, off:off + w],
                                    rrep1[:, off:off + w], ALU.mult)
            nc.vector.tensor_tensor(ln_b[:, c, off:off + w], t1[:, 0:w],
                                    nrep1[:, off:off + w], ALU.add)

    # =================== qkv projection ======================================
    qp = big.tile([128, NCH, AW], BF16, tag="qo", name="qp")
    kp = big.tile([128, NCH, KW], BF16, tag="kp", name="kp")
    vp = big.tile([128, NCH, AW], BF16, tag="vp", name="vp")
    nc.vector.memset(qp[:, :, A:AW], 0.0)
    nc.vector.memset(kp[:, :, 0:70], 0.0)
    nc.vector.memset(kp[:, :, 70 + A:KW], 0.0)

    for g in range(18):
        ps = psa() if g % 2 == 0 else psb()
        for s, (off, w) in enumerate(SEG_A):
            for c in range(NCH):
                nc.tensor.matmul(ps[:, s, 0:w],
                                 wq_t[:, c, 128 * g:128 * (g + 1)],
                                 ln_b[:, c, off:off + w],
                                 start=(c == 0), stop=(c == NCH - 1))
        if g < 6:
            dst = qp[:, g, :]
        elif g < 12:
            dst = kp[:, g - 6, 70:]
        else:
            dst = vp[:, g - 12, :]
        copy2("act", dst[:, 0:772], dst[:, 772:1156], ps, SEG_A,
              func=AF.Identity, bias=bqkv_t[:, g:g + 1])
    # vp pad must be zero for shifted o-products
    nc.vector.memset(vp[:, :, A:AW], 0.0)

    # =================== scores: 25 displacement maps ========================
    E0 = psa()                      # [(16e x 8h), a]
    E1 = psb()                      # [(9e x 8h), a]
    for c in range(NCH):
        for ei, (er, ec) in enumerate(E_LIST):
            grp, j = (0, ei) if ei < 16 else (1, ei - 16)
            koff = 70 + 34 * er + ec
            Eg = E0 if grp == 0 else E1
            m = 128 if grp == 0 else 72
            prod = prodp.tile([128, A], BF16, tag="pr", name="prod")
            if ei in POOL_EI:
                nc.gpsimd.tensor_tensor(prod, qp[:, c, 0:A],
                                        kp[:, c, koff:koff + A], ALU.mult)
            else:
                nc.vector.tensor_tensor(prod, qp[:, c, 0:A],
                                        kp[:, c, koff:koff + A], ALU.mult)
            lhs = selqk_t[:, c, 120 - 8 * j:120 - 8 * j + m]
            first = (c == 0 and j == 0)
            last = (c == NCH - 1 and j == (15 if grp == 0 else 8))
            for s, (off, w) in enumerate(SEG_A):
                nc.tensor.matmul(Eg[0:m, s, 0:w], lhs, prod[:, off:off + w],
                                 start=first, stop=last, skip_group_check=True)

    F0 = big.tile([128, AW], BF16, tag="F0", name="F0")
    F1 = big.tile([72, AW], BF16, tag="F1", name="F1")
    nc.vector.memset(F0[:, A:AW], 0.0)
    nc.vector.memset(F1[:, A:AW], 0.0)
    for Ft, Eg, m in ((F0, E0, 128), (F1, E1, 72)):
        copy2("act", Ft[:, 0:772], Ft[:, 772:1156], Eg, SEG_A, m=m,
              func=AF.Exp, scale=SCALE)

    # =================== denominators: G (x9) -> R = 1/(9G) ==================
    Gp = psa()
    for s, (off, w) in enumerate(SEG_A):
        nc.tensor.matmul(Gp[0:72, s, 0:w], gsel0_t, F0[:, off:off + w],
                         start=True, stop=False)
        nc.tensor.matmul(Gp[0:72, s, 0:w], gsel1_t, F1[0:72, off:off + w],
                         start=False, stop=True, skip_group_check=True)
    R = big.tile([72, AW], BF16, tag="R", name="R")
    nc.vector.memset(R[:, A:AW], 0.0)
    nc.vector.reciprocal(R[:, 0:772], Gp[0:72, 0:2, 0:386])
    nc.vector.reciprocal(R[:, 772:1156], Gp[0:72, 2, 0:384])

    # =================== W = sum_ki (wsel_ki @ F)(shift ski) * R_rep =========
    W_acc = big.tile([72, NW], BF16, tag="Wa", name="W_acc")
    for ki, (kir, kic) in enumerate(KI_LIST):
        base = 32 * min(ki // 4, 2)
        bsz = min(72 - base, 32)
        ski = 34 * kir + kic
        Bp = psa()   # B_ki = wsel @ F(shift)
        Rp = psb()   # rrep' = R row (shift) broadcast to 72 rows
        for s, (off, w) in enumerate(SEG_N):
            nc.tensor.matmul(Bp[0:72, s, 0:w], wsel0_t[:, ki, :],
                             F0[:, ski + off:ski + off + w],
                             start=True, stop=False)
            nc.tensor.matmul(Bp[0:72, s, 0:w], wsel1_t[0:72, ki, :],
                             F1[0:72, ski + off:ski + off + w],
                             start=False, stop=True, skip_group_check=True)
            nc.tensor.matmul(Rp[0:72, s, 0:w],
                             repsel32_t[base:base + bsz, ki, 0:72],
                             R[base:base + bsz, ski + off:ski + off + w],
                             start=True, stop=True)
        B_b = brp.tile([72, NW], BF16, tag="bb", name="B_b")
        copy2("act", B_b[:, 0:726], B_b[:, 726:1088], Bp, SEG_N, m=72)
        # (B * 1.0) * rrep'  -- STT reads rrep' straight from PSUM
        dst = W_acc if ki == 0 else brp.tile([72, NW], BF16, tag="wt",
                                             name="W_tmp")
        nc.vector.scalar_tensor_tensor(dst[:, 0:726], B_b[:, 0:726], 1.0,
                                       Rp[0:72, 0:2, 0:363],
                                       ALU.mult, ALU.mult)
        nc.vector.scalar_tensor_tensor(dst[:, 726:1088], B_b[:, 726:1088],
                                       1.0, Rp[0:72, 2, 0:362],
                                       ALU.mult, ALU.mult)
        if ki > 0:
            nc.vector.tensor_tensor(W_acc, W_acc, dst, ALU.add)


    # =================== o_mean accumulation ================================
    # v channels are 16-interleaved (head = p//16 in every chunk), so one
    # W-broadcast per kj serves all 6 chunks; materialize all 9 upfront.
    wr_all = big.tile([128, 9, NW], BF16, tag="kp", name="wr_all")
    for kj in range(9):
        base = 32 * min(kj // 4, 2)
        bsz = min(72 - base, 32)
        wps = psa() if kj % 2 == 0 else psb()
        for s_, (off, w) in enumerate(SEG_N):
            nc.tensor.matmul(wps[:, s_, 0:w],
                             csel32_t[base:base + bsz, kj, :],
                             W_acc[base:base + bsz, off:off + w],
                             start=True, stop=True)
        copy2("act", wr_all[:, kj, 0:726], wr_all[:, kj, 726:1088], wps, SEG_N)
    o_b = big.tile([128, NCH, NW], BF16, tag="qo", name="o_b")
    for c in range(NCH):
        oacc = psa() if c % 2 == 0 else psb()
        # kj 7,8 on Pool, issued first, consumed at the end of the chain
        pprods = []
        for kj in (7, 8):
            kjr, kjc = KI_LIST[kj]
            skj = 34 * kjr + kjc
            pp = brp.tile([128, NW], BF16, tag="pp", name="prodp_" + str(kj))
            nc.gpsimd.tensor_tensor(pp, vp[:, c, skj:skj + NW],
                                    wr_all[:, kj, :], ALU.mult)
            pprods.append(pp)
        for kj in range(7):
            kjr, kjc = KI_LIST[kj]
            skj = 34 * kjr + kjc
            prod = prodp.tile([128, NW], BF16, tag="pr", name="prodo")
            nc.vector.tensor_tensor(prod, vp[:, c, skj:skj + NW],
                                    wr_all[:, kj, :], ALU.mult)
            for s_, (off, w) in enumerate(SEG_N):
                nc.tensor.matmul(oacc[:, s_, 0:w], ident_t,
                                 prod[:, off:off + w],
                                 start=(kj == 0), stop=False,
                                 skip_group_check=True)
        for i2, kj in enumerate((7, 8)):
            for s_, (off, w) in enumerate(SEG_N):
                nc.tensor.matmul(oacc[:, s_, 0:w], ident_t,
                                 pprods[i2][:, off:off + w],
                                 start=False, stop=(kj == 8),
                                 skip_group_check=True)
        copy2("act", o_b[:, c, 0:726], o_b[:, c, 726:1088], oacc, SEG_N)

    # =================== proj + residual -> u ================================
    u_b = big.tile([128, NCH, NW], BF16, tag="ub", name="u_b")
    sq2_all = big.tile([128, NCH, NW], BF16, tag="vp", name="sq2_all")
    for g in range(NCH):
        ps = psa() if g % 2 == 0 else psb()
        for s, (off, w) in enumerate(SEG_N):
            for c in range(NCH):
                nc.tensor.matmul(ps[:, s, 0:w],
                                 wp_t[:, c, 128 * g:128 * (g + 1)],
                                 o_b[:, c, off:off + w],
                                 start=(c == 0), stop=(c == NCH - 1))
        # u = t9/9 + r, then + bproj
        nc.vector.scalar_tensor_tensor(u_b[:, g, 0:726], t9[:, g, 0:726],
                                       1.0 / 9.0, ps[:, 0:2, 0:363],
                                       ALU.mult, ALU.add)
        nc.vector.scalar_tensor_tensor(u_b[:, g, 726:1088], t9[:, g, 726:1088],
                                       1.0 / 9.0, ps[:, 2, 0:362],
                                       ALU.mult, ALU.add)
        nc.vector.tensor_scalar_add(u_b[:, g, :], u_b[:, g, :],
                                    bproj_t[:, g:g + 1])
        nc.scalar.activation(sq2_all[:, g, :], u_b[:, g, :], AF.Square)

    # =================== LayerNorm 2 ========================================
    stat1b = psa()
    stat2b = psb()
    for c in range(NCH):
        for s, (off, w) in enumerate(SEG_N):
            nc.tensor.matmul(stat1b[0:1, s, 0:w], onesk_t,
                             u_b[:, c, off:off + w],
                             start=(c == 0), stop=(c == NCH - 1),
                             skip_group_check=True)
            nc.tensor.matmul(stat2b[0:1, s, 0:w], onesk_t,
                             sq2_all[:, c, off:off + w],
                             start=(c == 0), stop=(c == NCH - 1),
                             skip_group_check=True)
    rstd2, nmur2 = ln_smalls(stat1b, stat2b, NW, SEG_N, "l2")
    rrep2 = small.tile([128, NW], BF16, tag="lnrr", name="rrep2")
    nrep2 = small.tile([128, NW], BF16, tag="lnnr", name="nrep2")
    rscr2 = drp.tile([1, NW], BF16, tag="scr", name="rscr2")
    nscr2 = drp.tile([1, NW], BF16, tag="scr", name="nscr2")
    nc.sync.dma_start(out=rscr2, in_=rstd2)
    nc.sync.dma_start(out=nscr2, in_=nmur2)
    nc.sync.dma_start(out=rrep2, in_=rscr2.to_broadcast([128, NW]))
    nc.sync.dma_start(out=nrep2, in_=nscr2.to_broadcast([128, NW]))
    ln2_b = big.tile([128, NCH, NW], BF16, tag="ln", name="ln2_b")
    for s_, (off, w) in enumerate(SEG_N):
        for c in range(NCH):
            t1 = prodp.tile([128, NW], BF16, tag="pr", name="t1b")
            nc.vector.tensor_tensor(t1[:, 0:w], u_b[:, c, off:off + w],
                                    rrep2[:, off:off + w], ALU.mult)
            nc.vector.tensor_tensor(ln2_b[:, c, off:off + w], t1[:, 0:w],
                                    nrep2[:, off:off + w], ALU.add)

    # =================== fc + relu + output =================================
    y_t = big.tile([128, 2, NW], F32, tag="kp", name="y_t")
    for g in range(2):
        ps = psa() if g % 2 == 0 else psb()
        for s, (off, w) in enumerate(SEG_N):
            for c in range(NCH):
                nc.tensor.matmul(ps[:, s, 0:w],
                                 wf_t[:, c, 128 * g:128 * (g + 1)],
                                 ln2_b[:, c, off:off + w],
                                 start=(c == 0), stop=(c == NCH - 1))
        copy2("act", y_t[:, g, 0:726], y_t[:, g, 726:1088], ps, SEG_N,
              func=AF.Relu, bias=bfc_t[:, g:g + 1])
        src = y_t[:, g, :].rearrange("p (r c) -> p r c", c=34)[:, :, 0:32]
        nc.sync.dma_start(out=y_d[g], in_=src)


# ============================ host-side wrapper =============================

def _build_sels():
    bf = ml_dtypes.bfloat16
    selqk = np.zeros((NCH, 128, 248), np.float32)
    for c in range(NCH):
        for r in range(128):
            h = (128 * c + r) // HD
            selqk[c, r, 120 + h] = 1.0
    gsel0 = np.zeros((128, 72), np.float32)
    gsel1 = np.zeros((72, 72), np.float32)
    for ki, (kir, kic) in enumerate(KI_LIST):
        for j, (er, ec) in enumerate(E_LIST):
            if -kir <= er <= 2 - kir and -kic <= ec <= 2 - kic:
                for h in range(HEADS):
                    if j < 16:
                        gsel0[8 * j + h, 8 * ki + h] = 9.0
                    else:
                        gsel1[8 * (j - 16) + h, 8 * ki + h] = 9.0
    wsel0 = np.zeros((128, 9, 72), np.float32)
    wsel1 = np.zeros((72, 9, 72), np.float32)
    for ki, (kir, kic) in enumerate(KI_LIST):
        for j, (er, ec) in enumerate(E_LIST):
            kjr, kjc = er + kir, ec + kic
            if 0 <= kjr <= 2 and 0 <= kjc <= 2:
                kj = 3 * kjr + kjc
                for h in range(HEADS):
                    if j < 16:
                        wsel0[8 * j + h, ki, 8 * kj + h] = 1.0
                    else:
                        wsel1[8 * (j - 16) + h, ki, 8 * kj + h] = 1.0
    repsel32 = np.zeros((72, 9, 72), np.float32)
    csel32 = np.zeros((72, 9, 128), np.float32)
    for ki in range(9):
        for h in range(HEADS):
            for col in range(72):
                if col % 8 == h:
                    repsel32[8 * ki + h, ki, col] = 1.0
            for p in range(128):
                if p // 16 == h:
                    csel32[8 * ki + h, ki, p] = 1.0
    ident = np.eye(128, dtype=np.float32)
    onesk = np.ones((128, 1), np.float32)
    out = dict(selqk=selqk, gsel0=gsel0, gsel1=gsel1, wsel0=wsel0, wsel1=wsel1,
               repsel32=repsel32, csel32=csel32, ident=ident, onesk=onesk)
    return {k: v.astype(bf) for k, v in out.items()}


@functools.lru_cache(maxsize=1)
def _build_module():
    nc = bacc.Bacc("TRN2", target_bir_lowering=False, debug=False)
    ins = {}

    def din(name, shape, dt):
        ins[name] = nc.dram_tensor(name, shape, dt, kind="ExternalInput").ap()

    din("xp", [NCH, 128, A], BF16)
    din("wqkv", [NCH, 128, 2304], BF16)
    din("wproj", [NCH, 128, 768], BF16)
    din("wfc", [NCH, 128, 256], BF16)
    din("bqkv", [128, 18], F32)
    din("bproj", [128, NCH], F32)
    din("bfc", [128, 2], F32)
    din("selqk", [NCH, 128, 248], BF16)
    din("gsel0", [128, 72], BF16)
    din("gsel1", [72, 72], BF16)
    din("wsel0", [128, 9, 72], BF16)
    din("wsel1", [72, 9, 72], BF16)
    din("repsel32", [72, 9, 72], BF16)
    din("csel32", [72, 9, 128], BF16)
    din("ident", [128, 128], BF16)
    din("onesk", [128, 1], BF16)
    outs = {"y": nc.dram_tensor("y", [2, 128, 32, 32], F32,
                                kind="ExternalOutput").ap()}

    from contextlib import ExitStack
    with tile.TileContext(nc) as tc:
        with ExitStack() as ctx:
            with nc.allow_low_precision(reason="bf16 kernel by design"):
                emit_kernel(ctx, tc, ins, outs)
    nc.compile()
    return nc


def kernel(x, w_qkv, b_qkv, w_proj, b_proj, g1, beta1, g2, beta2, w_fc, b_fc,
           _run_kwargs=None):
    bf = ml_dtypes.bfloat16
    x = np.asarray(x, np.float32)
    B = x.shape[0]
    assert x.shape == (8, C, 32, 32)

    w_qkv = np.asarray(w_qkv, np.float32)
    b_qkv = np.asarray(b_qkv, np.float32)
    w_fc = np.asarray(w_fc, np.float32)
    b_fc = np.asarray(b_fc, np.float32)
    g1 = np.asarray(g1, np.float32)
    beta1 = np.asarray(beta1, np.float32)
    g2 = np.asarray(g2, np.float32)
    beta2 = np.asarray(beta2, np.float32)
    wq = g1[:, None] * w_qkv
    bq = beta1 @ w_qkv + b_qkv
    # 16-interleave v out-channels (head = p//16 within every 128-chunk) and
    # permute proj rows to match
    old_of_new = np.array([96 * ((n % 128) // 16) + 16 * (n // 128)
                           + (n % 16) for n in range(C)])
    wq[:, 1536:] = wq[:, 1536 + old_of_new]
    bq[1536:] = bq[1536 + old_of_new]
    w_proj = np.asarray(w_proj, np.float32)[old_of_new, :]
    wf = g2[:, None] * w_fc
    bfc2 = beta2 @ w_fc + b_fc

    sels = _build_sels()
    shared = dict(
        wqkv=np.ascontiguousarray(wq.reshape(NCH, 128, 2304)).astype(bf),
        wproj=np.ascontiguousarray(
            np.asarray(w_proj, np.float32).reshape(NCH, 128, 768)).astype(bf),
        wfc=np.ascontiguousarray(wf.reshape(NCH, 128, 256)).astype(bf),
        bqkv=np.ascontiguousarray(bq.reshape(18, 128).T),
        bproj=np.ascontiguousarray(
            np.asarray(b_proj, np.float32).reshape(NCH, 128).T),
        bfc=np.ascontiguousarray(bfc2.reshape(2, 128).T),
        **sels,
    )
    in_maps = []
    for b in range(B):
        xpad = np.pad(x[b], ((0, 0), (1, 1), (1, 1)), mode="edge")
        xp = np.ascontiguousarray(xpad.reshape(NCH, 128, A)).astype(bf)
        in_maps.append(dict(xp=xp, **shared))

    nc = _build_module()
    res = run_bass_kernel_spmd(nc, in_maps, core_ids=list(range(8)),
                               **(_run_kwargs or {}))
    outs = []
    for b in range(B):
        y = np.asarray(res.results[b]["y"], np.float32)  # [2,128,32,32]
        outs.append(y.reshape(256, 32, 32))
    out = np.stack(outs).astype(np.float32)
    if _run_kwargs is not None:
        kernel.last_result = res
    return out


# revision 7
# speedup vs baseline: 1.1004x; 1.0108x over previous
"""Trainium2 Bass kernel for nn_AttnBlock (sparse 3x3-window attention block).

Structure (per core, one batch image):
  - LN1/qkv computed once per padded pixel (34x34 grid); g1/beta1 folded into
    w_qkv/b_qkv and g2/beta2 into w_fc/b_fc on the host, so LN applies are two
    plain DVE ops.
  - Scores deduplicated into 25 displacement maps E_e = q . shift_e(k)
    (DVE/Pool elementwise product + PE per-head column reduction).
  - Softmax denominators via box-sum selection matmul (gsel, scaled x9 to fold
    the window mean), W = sum_ki (wsel_ki @ F)(shift) * R-broadcast.
  - o_mean = sum_kj shift(v * W-broadcast); broadcasts via PE selection
    matmuls, PSUM->SBUF bf16 copies on Act/Pool so big DVE products run in
    2x (16-bit) mode, full width.
  - Box filter (residual mean) runs on the Pool engine under the qkv phase.

Sharding: data-parallel over batch B=8 -> one batch per NeuronCore.
"""

import functools
import numpy as np
import ml_dtypes

import concourse.bass as bass
import concourse.mybir as mybir
import concourse.tile as tile
from concourse import bacc
from concourse.bass_utils import run_bass_kernel_spmd

F32 = mybir.dt.float32
BF16 = mybir.dt.bfloat16
AF = mybir.ActivationFunctionType
ALU = mybir.AluOpType

C = 768
NCH = 6          # channel chunks of 128
G = 34           # padded grid side
A = G * G        # 1156 padded pixels
AW = 1160        # padded-pixel width with 4 pad cols
NW = 1088        # window-grid width = 32*34 (rows 0..31, cols 0..33)
KW = 1300        # k map width with +-70 margins (content at 70)
HEADS = 8
HD = 96
SCALE = HD ** -0.5
EPS = 1e-5

SEG_A = [(0, 386), (386, 386), (772, 384)]   # PSUM segs over the a-grid
SEG_N = [(0, 363), (363, 363), (726, 362)]   # PSUM segs over the n-grid

E_LIST = [(er, ec) for er in range(-2, 3) for ec in range(-2, 3)]  # 25
KI_LIST = [(r, c) for r in range(3) for c in range(3)]             # 9

POOL_EI = {4, 9, 14, 19, 24}  # displacement products offloaded to Pool


def emit_kernel(ctx, tc, ins, outs):
    nc = tc.nc
    xp_d = ins["xp"]          # [6,128,1156] bf16
    wq_d = ins["wqkv"]        # [6,128,2304] bf16 (g1-folded)
    wp_d = ins["wproj"]       # [6,128,768] bf16
    wf_d = ins["wfc"]         # [6,128,256] bf16 (g2-folded)
    bqkv_d = ins["bqkv"]      # [128,18] f32 (beta1-folded)
    bproj_d = ins["bproj"]    # [128,6] f32
    bfc_d = ins["bfc"]        # [128,2] f32 (beta2-folded)
    selqk_d = ins["selqk"]    # [6,128,248] bf16
    gsel0_d = ins["gsel0"]    # [128,72] bf16 (x9)
    gsel1_d = ins["gsel1"]    # [72,72] bf16 (x9)
    wsel0_d = ins["wsel0"]    # [128,9,72] bf16
    wsel1_d = ins["wsel1"]    # [72,9,72] bf16
    repsel9_d = ins["repsel32"]  # [72,9,72] bf16
    csel_d = ins["csel32"]    # [72,9,128] bf16
    ident_d = ins["ident"]    # [128,128] bf16
    onesk_d = ins["onesk"]    # [128,1] bf16
    y_d = outs["y"]           # [2,128,32,32] f32

    consts = ctx.enter_context(tc.tile_pool(name="consts", bufs=1))
    big = ctx.enter_context(tc.tile_pool(name="big", bufs=1))
    prodp = ctx.enter_context(tc.tile_pool(name="prodp", bufs=6))
    brp = ctx.enter_context(tc.tile_pool(name="brp", bufs=2))
    small = ctx.enter_context(tc.tile_pool(name="small", bufs=1))
    psA = ctx.enter_context(tc.tile_pool(name="psA", bufs=1, space="PSUM"))
    psB = ctx.enter_context(tc.tile_pool(name="psB", bufs=1, space="PSUM"))
    psC = ctx.enter_context(tc.tile_pool(name="psC", bufs=2, space="PSUM"))
    drp = ctx.enter_context(tc.tile_pool(name="drp", bufs=2, space="DRAM"))

    def psa():
        return psA.tile([128, 3, 512], F32, tag="a", name="psa_t")

    def psb():
        return psB.tile([128, 3, 512], F32, tag="b", name="psb_t")

    # two-op PSUM->SBUF copy helpers (banks 0-1 fused, bank 2), SEG-uniform
    def copy2(eng, dst01, dst2, ps, segs, m=128, func=AF.Copy, bias=None,
              scale=1.0):
        w01, w2 = segs[0][1], segs[2][1]
        if eng == "act":
            kw = {} if bias is None else {"bias": bias}
            nc.scalar.activation(dst01[0:m], ps[0:m, 0:2, 0:w01], func,
                                 scale=scale, **kw)
            nc.scalar.activation(dst2[0:m], ps[0:m, 2, 0:w2], func,
                                 scale=scale, **kw)
        elif eng == "pool":
            nc.gpsimd.tensor_copy(dst01[0:m], ps[0:m, 0:2, 0:w01])
            nc.gpsimd.tensor_copy(dst2[0:m], ps[0:m, 2, 0:w2])
        else:
            nc.vector.tensor_copy(dst01[0:m], ps[0:m, 0:2, 0:w01])
            nc.vector.tensor_copy(dst2[0:m], ps[0:m, 2, 0:w2])

    # ---- load constants ----
    def load(pool, name, shape, dt, src, tag=None):
        t = pool.tile(shape, dt, tag=tag or name, name=name)
        nc.sync.dma_start(out=t, in_=src)
        return t

    xpb = big.tile([128, NCH, AW], BF16, tag="bigA", name="xpb")
    onesk_t = load(consts, "onesk", [128, 1], BF16, onesk_d)
    for c in range(NCH):
        nc.sync.dma_start(out=xpb[:, c, 0:A], in_=xp_d[c])
    nc.vector.memset(xpb[:, :, A:AW], 0.0)
    wq_t = consts.tile([128, NCH, 2304], BF16, tag="wq", name="wq_t")
    wp_t = consts.tile([128, NCH, 768], BF16, tag="wp", name="wp_t")
    wf_t = consts.tile([128, NCH, 256], BF16, tag="wf", name="wf_t")
    selqk_t = consts.tile([128, NCH, 248], BF16, tag="selqk", name="selqk_t")
    for c in range(NCH):
        nc.sync.dma_start(out=wq_t[:, c, :], in_=wq_d[c])
        nc.sync.dma_start(out=wp_t[:, c, :], in_=wp_d[c])
        nc.sync.dma_start(out=wf_t[:, c, :], in_=wf_d[c])
        nc.sync.dma_start(out=selqk_t[:, c, :], in_=selqk_d[c])
    gsel0_t = load(consts, "gsel0", [128, 72], BF16, gsel0_d)
    gsel1_t = load(consts, "gsel1", [72, 72], BF16, gsel1_d)
    wsel0_t = load(consts, "wsel0", [128, 9, 72], BF16, wsel0_d)
    wsel1_t = load(consts, "wsel1", [72, 9, 72], BF16, wsel1_d)
    repsel32_t = load(consts, "repsel32", [72, 9, 72], BF16,
                      repsel9_d)
    csel32_t = load(consts, "csel32", [72, 9, 128], BF16, csel_d)
    ident_t = load(consts, "ident", [128, 128], BF16, ident_d)
    bqkv_t = load(small, "bqkv", [128, 18], F32, bqkv_d)
    bproj_t = load(small, "bproj", [128, NCH], F32, bproj_d)
    bfc_t = load(small, "bfc", [128, 2], F32, bfc_d)

    # =================== box filter on Pool (residual t_mean, x9) ============
    t9 = big.tile([128, NCH, NW], BF16, tag="t9", name="t9")
    for c in range(NCH):
        tr = brp.tile([128, 1158], BF16, tag="wr", name="tr")
        nc.gpsimd.tensor_tensor(tr, xpb[:, c, 0:1158], xpb[:, c, 1:1159],
                                ALU.add)
        nc.gpsimd.tensor_tensor(tr, tr, xpb[:, c, 2:1160], ALU.add)
        nc.gpsimd.tensor_tensor(t9[:, c, :], tr[:, 0:NW], tr[:, 34:34 + NW],
                                ALU.add)
        nc.gpsimd.tensor_tensor(t9[:, c, :], t9[:, c, :], tr[:, 68:68 + NW],
                                ALU.add)

    # =================== LayerNorm 1 (stats over channels via PE) ============
    stat1 = psa()   # sum x   [1, a]
    stat2 = psb()   # sum x^2 [1, a]
    for c in range(NCH):
        sqx = prodp.tile([128, A], BF16, tag="pr", name="sqx")
        nc.vector.tensor_tensor(sqx, xpb[:, c, 0:A], xpb[:, c, 0:A], ALU.mult)
        for s, (off, w) in enumerate(SEG_A):
            nc.tensor.matmul(stat1[0:1, s, 0:w], onesk_t,
                             xpb[:, c, off:off + w],
                             start=(c == 0), stop=(c == NCH - 1),
                             skip_group_check=True)
            nc.tensor.matmul(stat2[0:1, s, 0:w], onesk_t,
                             sqx[:, off:off + w],
                             start=(c == 0), stop=(c == NCH - 1),
                             skip_group_check=True)

    def ln_smalls(stat1, stat2, width, segs, tagpfx):
        """From PSUM sums -> rstd (bf16) and -mu*rstd (bf16), [1, width]."""
        ta = small.tile([1, width], F32, tag="lnta", name=tagpfx + "ta")
        xs = small.tile([1, width], F32, tag="lnxs", name=tagpfx + "xs")
        sq = small.tile([1, width], F32, tag="lnta", name=tagpfx + "sv")
        rstd = small.tile([1, width], BF16, tag="lnrs", name=tagpfx + "rs")
        nmur = small.tile([1, width], BF16, tag="lnnm", name=tagpfx + "nm")
        eps_t = small.tile([1, 1], F32, tag="lnep", name=tagpfx + "ep")
        nc.vector.memset(eps_t, EPS)
        for s, (off, w) in enumerate(segs):
            nc.scalar.activation(ta[:, off:off + w], stat1[0:1, s, 0:w],
                                 AF.Square)
            # xs = ta/768 - stat2   (= -768*var)
            nc.vector.scalar_tensor_tensor(xs[:, off:off + w],
                                           ta[:, off:off + w], 1.0 / C,
                                           stat2[0:1, s, 0:w],
                                           ALU.mult, ALU.subtract)
        # sq = sqrt(xs * (-1/768) + eps) = sqrt(var + eps)
        for s, (off, w) in enumerate(segs):
            nc.scalar.activation(sq[:, off:off + w], xs[:, off:off + w],
                                 AF.Sqrt, bias=eps_t, scale=-1.0 / C)
            nc.vector.reciprocal(rstd[:, off:off + w], sq[:, off:off + w])
        # nmur = (stat1 * -1/768) * rstd = -mu * rstd  (stat1 still live: the
        # next user of its PSUM tile is WAR-fenced behind this read)
        for s, (off, w) in enumerate(segs):
            nc.vector.scalar_tensor_tensor(nmur[:, off:off + w],
                                           stat1[0:1, s, 0:w], -1.0 / C,
                                           rstd[:, off:off + w],
                                           ALU.mult, ALU.mult)
        return rstd, nmur

    rstd1, nmur1 = ln_smalls(stat1, stat2, A, SEG_A, "l1")

    # broadcast rstd / nmur to 128 partitions via partition-step-0 DMA
    rrep1 = small.tile([128, A], BF16, tag="lnrr", name="rrep1")
    nrep1 = small.tile([128, A], BF16, tag="lnnr", name="nrep1")
    rscr1 = drp.tile([1, A], BF16, tag="scr", name="rscr1")
    nscr1 = drp.tile([1, A], BF16, tag="scr", name="nscr1")
    nc.sync.dma_start(out=rscr1, in_=rstd1)
    nc.sync.dma_start(out=nscr1, in_=nmur1)
    nc.sync.dma_start(out=rrep1, in_=rscr1.to_broadcast([128, A]))
    nc.sync.dma_start(out=nrep1, in_=nscr1.to_broadcast([128, A]))

    ln_b = big.tile([128, NCH, A], BF16, tag="ln", name="ln_b")
    for s_, (off, w) in enumerate(SEG_A):
        for c in range(NCH):
            t1 = prodp.tile([128, A], BF16, tag="pr", name="t1")
            nc.vector.tensor_tensor(t1[:, 0:w], xpb[:, c, off:off + w],
                                    rrep1[:, off:off + w], ALU.mult)
            nc.vector.tensor_tensor(ln_b[:, c, off:off + w], t1[:, 0:w],
                                    nrep1[:, off:off + w], ALU.add)

    # =================== qkv projection ======================================
    qp = big.tile([128, NCH, AW], BF16, tag="qo", name="qp")
    kp = big.tile([128, NCH, KW], BF16, tag="kp", name="kp")
    vp = big.tile([128, NCH, AW], BF16, tag="vp", name="vp")
    nc.vector.memset(qp[:, :, A:AW], 0.0)
    nc.vector.memset(kp[:, :, 0:70], 0.0)
    nc.vector.memset(kp[:, :, 70 + A:KW], 0.0)

    for g in range(18):
        ps = psa() if g % 2 == 0 else psb()
        for s, (off, w) in enumerate(SEG_A):
            for c in range(NCH):
                nc.tensor.matmul(ps[:, s, 0:w],
                                 wq_t[:, c, 128 * g:128 * (g + 1)],
                                 ln_b[:, c, off:off + w],
                                 start=(c == 0), stop=(c == NCH - 1))
        if g < 6:
            dst = qp[:, g, :]
        elif g < 12:
            dst = kp[:, g - 6, 70:]
        else:
            dst = vp[:, g - 12, :]
        copy2("act", dst[:, 0:772], dst[:, 772:1156], ps, SEG_A,
              func=AF.Identity, bias=bqkv_t[:, g:g + 1])
    # vp pad must be zero for shifted o-products
    nc.vector.memset(vp[:, :, A:AW], 0.0)

    # =================== scores: 25 displacement maps ========================
    E0 = psa()                      # [(16e x 8h), a]
    E1 = psb()                      # [(9e x 8h), a]
    for c in range(NCH):
        for ei, (er, ec) in enumerate(E_LIST):
            grp, j = (0, ei) if ei < 16 else (1, ei - 16)
            koff = 70 + 34 * er + ec
            Eg = E0 if grp == 0 else E1
            m = 128 if grp == 0 else 72
            prod = prodp.tile([128, A], BF16, tag="pr", name="prod")
            if ei in POOL_EI:
                nc.gpsimd.tensor_tensor(prod, qp[:, c, 0:A],
                                        kp[:, c, koff:koff + A], ALU.mult)
            else:
                nc.vector.tensor_tensor(prod, qp[:, c, 0:A],
                                        kp[:, c, koff:koff + A], ALU.mult)
            lhs = selqk_t[:, c, 120 - 8 * j:120 - 8 * j + m]
            first = (c == 0 and j == 0)
            last = (c == NCH - 1 and j == (15 if grp == 0 else 8))
            for s, (off, w) in enumerate(SEG_A):
                nc.tensor.matmul(Eg[0:m, s, 0:w], lhs, prod[:, off:off + w],
                                 start=first, stop=last, skip_group_check=True)

    F0 = big.tile([128, AW], BF16, tag="F0", name="F0")
    F1 = big.tile([72, AW], BF16, tag="F1", name="F1")
    nc.vector.memset(F0[:, A:AW], 0.0)
    nc.vector.memset(F1[:, A:AW], 0.0)
    for Ft, Eg, m in ((F0, E0, 128), (F1, E1, 72)):
        copy2("act", Ft[:, 0:772], Ft[:, 772:1156], Eg, SEG_A, m=m,
              func=AF.Exp, scale=SCALE)

    # =================== denominators: G (x9) -> R = 1/(9G) ==================
    Gp = psa()
    for s, (off, w) in enumerate(SEG_A):
        nc.tensor.matmul(Gp[0:72, s, 0:w], gsel0_t, F0[:, off:off + w],
                         start=True, stop=False)
        nc.tensor.matmul(Gp[0:72, s, 0:w], gsel1_t, F1[0:72, off:off + w],
                         start=False, stop=True, skip_group_check=True)
    R = big.tile([72, AW], BF16, tag="R", name="R")
    nc.vector.memset(R[:, A:AW], 0.0)
    nc.vector.reciprocal(R[:, 0:772], Gp[0:72, 0:2, 0:386])
    nc.vector.reciprocal(R[:, 772:1156], Gp[0:72, 2, 0:384])

    # =================== W = sum_ki (wsel_ki @ F)(shift ski) * R_rep =========
    W_acc = big.tile([72, NW], BF16, tag="Wa", name="W_acc")
    for ki, (kir, kic) in enumerate(KI_LIST):
        base = 32 * min(ki // 4, 2)
        bsz = min(72 - base, 32)
        ski = 34 * kir + kic
        Bp = psa()   # B_ki = wsel @ F(shift)
        Rp = psb()   # rrep' = R row (shift) broadcast to 72 rows
        for s, (off, w) in enumerate(SEG_N):
            nc.tensor.matmul(Bp[0:72, s, 0:w], wsel0_t[:, ki, :],
                             F0[:, ski + off:ski + off + w],
                             start=True, stop=False)
            nc.tensor.matmul(Bp[0:72, s, 0:w], wsel1_t[0:72, ki, :],
                             F1[0:72, ski + off:ski + off + w],
                             start=False, stop=True, skip_group_check=True)
            nc.tensor.matmul(Rp[0:72, s, 0:w],
                             repsel32_t[base:base + bsz, ki, 0:72],
                             R[base:base + bsz, ski + off:ski + off + w],
                             start=True, stop=True)
        B_b = brp.tile([72, NW], BF16, tag="bb", name="B_b")
        copy2("act", B_b[:, 0:726], B_b[:, 726:1088], Bp, SEG_N, m=72)
        # (B * 1.0) * rrep'  -- STT reads rrep' straight from PSUM
        dst = W_acc if ki == 0 else brp.tile([72, NW], BF16, tag="wt",
                                             name="W_tmp")
        nc.vector.scalar_tensor_tensor(dst[:, 0:726], B_b[:, 0:726], 1.0,
                                       Rp[0:72, 0:2, 0:363],
                                       ALU.mult, ALU.mult)
        nc.vector.scalar_tensor_tensor(dst[:, 726:1088], B_b[:, 726:1088],
                                       1.0, Rp[0:72, 2, 0:362],
                                       ALU.mult, ALU.mult)
        if ki > 0:
            nc.vector.tensor_tensor(W_acc, W_acc, dst, ALU.add)


    # =================== o_mean accumulation ================================
    # v channels are 16-interleaved (head = p//16 in every chunk), so one
    # W-broadcast per kj serves all 6 chunks; materialize all 9 upfront.
    wr_all = big.tile([128, 9, NW], BF16, tag="kp", name="wr_all")
    for kj in range(9):
        base = 32 * min(kj // 4, 2)
        bsz = min(72 - base, 32)
        wps = psa() if kj % 2 == 0 else psb()
        for s_, (off, w) in enumerate(SEG_N):
            nc.tensor.matmul(wps[:, s_, 0:w],
                             csel32_t[base:base + bsz, kj, :],
                             W_acc[base:base + bsz, off:off + w],
                             start=True, stop=True)
        nc.scalar.activation(wr_all[:, kj, 0:726], wps[:, 0:2, 0:363],
                             AF.Copy)
        nc.vector.tensor_copy(wr_all[:, kj, 726:1088], wps[:, 2, 0:362])
    o_b = big.tile([128, NCH, NW], BF16, tag="qo", name="o_b")
    for c in range(NCH):
        oacc = psa() if c % 2 == 0 else psb()
        # kj 7,8 on Pool, issued first, consumed at the end of the chain
        pprods = []
        for kj in (7, 8):
            kjr, kjc = KI_LIST[kj]
            skj = 34 * kjr + kjc
            pp = brp.tile([128, NW], BF16, tag="pp", name="prodp_" + str(kj))
            nc.gpsimd.tensor_tensor(pp, vp[:, c, skj:skj + NW],
                                    wr_all[:, kj, :], ALU.mult)
            pprods.append(pp)
        for kj in range(7):
            kjr, kjc = KI_LIST[kj]
            skj = 34 * kjr + kjc
            prod = prodp.tile([128, NW], BF16, tag="pr", name="prodo")
            nc.vector.tensor_tensor(prod, vp[:, c, skj:skj + NW],
                                    wr_all[:, kj, :], ALU.mult)
            for s_, (off, w) in enumerate(SEG_N):
                nc.tensor.matmul(oacc[:, s_, 0:w], ident_t,
                                 prod[:, off:off + w],
                                 start=(kj == 0), stop=False,
                                 skip_group_check=True)
        for i2, kj in enumerate((7, 8)):
            for s_, (off, w) in enumerate(SEG_N):
                nc.tensor.matmul(oacc[:, s_, 0:w], ident_t,
                                 pprods[i2][:, off:off + w],
                                 start=False, stop=(kj == 8),
                                 skip_group_check=True)
        copy2("act", o_b[:, c, 0:726], o_b[:, c, 726:1088], oacc, SEG_N)

    # =================== proj + residual -> u ================================
    u_b = big.tile([128, NCH, NW], BF16, tag="ub", name="u_b")
    sq2_all = big.tile([128, NCH, NW], BF16, tag="vp", name="sq2_all")
    for g in range(NCH):
        ps = psa() if g % 2 == 0 else psb()
        for s, (off, w) in enumerate(SEG_N):
            for c in range(NCH):
                nc.tensor.matmul(ps[:, s, 0:w],
                                 wp_t[:, c, 128 * g:128 * (g + 1)],
                                 o_b[:, c, off:off + w],
                                 start=(c == 0), stop=(c == NCH - 1))
        # u = t9/9 + r, then + bproj
        nc.vector.scalar_tensor_tensor(u_b[:, g, 0:726], t9[:, g, 0:726],
                                       1.0 / 9.0, ps[:, 0:2, 0:363],
                                       ALU.mult, ALU.add)
        nc.vector.scalar_tensor_tensor(u_b[:, g, 726:1088], t9[:, g, 726:1088],
                                       1.0 / 9.0, ps[:, 2, 0:362],
                                       ALU.mult, ALU.add)
        nc.vector.tensor_scalar_add(u_b[:, g, :], u_b[:, g, :],
                                    bproj_t[:, g:g + 1])
        nc.scalar.activation(sq2_all[:, g, :], u_b[:, g, :], AF.Square)

    # =================== LayerNorm 2 ========================================
    stat1b = psa()
    stat2b = psb()
    for c in range(NCH):
        for s, (off, w) in enumerate(SEG_N):
            nc.tensor.matmul(stat1b[0:1, s, 0:w], onesk_t,
                             u_b[:, c, off:off + w],
                             start=(c == 0), stop=(c == NCH - 1),
                             skip_group_check=True)
            nc.tensor.matmul(stat2b[0:1, s, 0:w], onesk_t,
                             sq2_all[:, c, off:off + w],
                             start=(c == 0), stop=(c == NCH - 1),
                             skip_group_check=True)
    rstd2, nmur2 = ln_smalls(stat1b, stat2b, NW, SEG_N, "l2")
    rrep2 = small.tile([128, NW], BF16, tag="lnrr", name="rrep2")
    nrep2 = small.tile([128, NW], BF16, tag="lnnr", name="nrep2")
    rscr2 = drp.tile([1, NW], BF16, tag="scr", name="rscr2")
    nscr2 = drp.tile([1, NW], BF16, tag="scr", name="nscr2")
    nc.sync.dma_start(out=rscr2, in_=rstd2)
    nc.sync.dma_start(out=nscr2, in_=nmur2)
    nc.sync.dma_start(out=rrep2, in_=rscr2.to_broadcast([128, NW]))
    nc.sync.dma_start(out=nrep2, in_=nscr2.to_broadcast([128, NW]))
    ln2_b = big.tile([128, NCH, NW], BF16, tag="ln", name="ln2_b")
    for s_, (off, w) in enumerate(SEG_N):
        for c in range(NCH):
            eng = nc.gpsimd if c == 5 else nc.vector
            t1 = prodp.tile([128, NW], BF16, tag="pr", name="t1b")
            eng.tensor_tensor(t1[:, 0:w], u_b[:, c, off:off + w],
                              rrep2[:, off:off + w], ALU.mult)
            eng.tensor_tensor(ln2_b[:, c, off:off + w], t1[:, 0:w],
                              nrep2[:, off:off + w], ALU.add)

    # =================== fc + relu + output =================================
    y_t = big.tile([128, 2, NW], F32, tag="kp", name="y_t")
    for g in range(2):
        ps = psa() if g % 2 == 0 else psb()
        for s, (off, w) in enumerate(SEG_N):
            for c in range(NCH):
                nc.tensor.matmul(ps[:, s, 0:w],
                                 wf_t[:, c, 128 * g:128 * (g + 1)],
                                 ln2_b[:, c, off:off + w],
                                 start=(c == 0), stop=(c == NCH - 1))
        copy2("act", y_t[:, g, 0:726], y_t[:, g, 726:1088], ps, SEG_N,
              func=AF.Relu, bias=bfc_t[:, g:g + 1])
        src = y_t[:, g, :].rearrange("p (r c) -> p r c", c=34)[:, :, 0:32]
        nc.sync.dma_start(out=y_d[g], in_=src)


# ============================ host-side wrapper =============================

def _build_sels():
    bf = ml_dtypes.bfloat16
    selqk = np.zeros((NCH, 128, 248), np.float32)
    for c in range(NCH):
        for r in range(128):
            h = (128 * c + r) // HD
            selqk[c, r, 120 + h] = 1.0
    gsel0 = np.zeros((128, 72), np.float32)
    gsel1 = np.zeros((72, 72), np.float32)
    for ki, (kir, kic) in enumerate(KI_LIST):
        for j, (er, ec) in enumerate(E_LIST):
            if -kir <= er <= 2 - kir and -kic <= ec <= 2 - kic:
                for h in range(HEADS):
                    if j < 16:
                        gsel0[8 * j + h, 8 * ki + h] = 9.0
                    else:
                        gsel1[8 * (j - 16) + h, 8 * ki + h] = 9.0
    wsel0 = np.zeros((128, 9, 72), np.float32)
    wsel1 = np.zeros((72, 9, 72), np.float32)
    for ki, (kir, kic) in enumerate(KI_LIST):
        for j, (er, ec) in enumerate(E_LIST):
            kjr, kjc = er + kir, ec + kic
            if 0 <= kjr <= 2 and 0 <= kjc <= 2:
                kj = 3 * kjr + kjc
                for h in range(HEADS):
                    if j < 16:
                        wsel0[8 * j + h, ki, 8 * kj + h] = 1.0
                    else:
                        wsel1[8 * (j - 16) + h, ki, 8 * kj + h] = 1.0
    repsel32 = np.zeros((72, 9, 72), np.float32)
    csel32 = np.zeros((72, 9, 128), np.float32)
    for ki in range(9):
        for h in range(HEADS):
            for col in range(72):
                if col % 8 == h:
                    repsel32[8 * ki + h, ki, col] = 1.0
            for p in range(128):
                if p // 16 == h:
                    csel32[8 * ki + h, ki, p] = 1.0
    ident = np.eye(128, dtype=np.float32)
    onesk = np.ones((128, 1), np.float32)
    out = dict(selqk=selqk, gsel0=gsel0, gsel1=gsel1, wsel0=wsel0, wsel1=wsel1,
               repsel32=repsel32, csel32=csel32, ident=ident, onesk=onesk)
    return {k: v.astype(bf) for k, v in out.items()}


@functools.lru_cache(maxsize=1)
def _build_module():
    nc = bacc.Bacc("TRN2", target_bir_lowering=False, debug=False)
    ins = {}

    def din(name, shape, dt):
        ins[name] = nc.dram_tensor(name, shape, dt, kind="ExternalInput").ap()

    din("xp", [NCH, 128, A], BF16)
    din("wqkv", [NCH, 128, 2304], BF16)
    din("wproj", [NCH, 128, 768], BF16)
    din("wfc", [NCH, 128, 256], BF16)
    din("bqkv", [128, 18], F32)
    din("bproj", [128, NCH], F32)
    din("bfc", [128, 2], F32)
    din("selqk", [NCH, 128, 248], BF16)
    din("gsel0", [128, 72], BF16)
    din("gsel1", [72, 72], BF16)
    din("wsel0", [128, 9, 72], BF16)
    din("wsel1", [72, 9, 72], BF16)
    din("repsel32", [72, 9, 72], BF16)
    din("csel32", [72, 9, 128], BF16)
    din("ident", [128, 128], BF16)
    din("onesk", [128, 1], BF16)
    outs = {"y": nc.dram_tensor("y", [2, 128, 32, 32], F32,
                                kind="ExternalOutput").ap()}

    from contextlib import ExitStack
    with tile.TileContext(nc) as tc:
        with ExitStack() as ctx:
            with nc.allow_low_precision(reason="bf16 kernel by design"):
                emit_kernel(ctx, tc, ins, outs)
    nc.compile()
    return nc


def kernel(x, w_qkv, b_qkv, w_proj, b_proj, g1, beta1, g2, beta2, w_fc, b_fc,
           _run_kwargs=None):
    bf = ml_dtypes.bfloat16
    x = np.asarray(x, np.float32)
    B = x.shape[0]
    assert x.shape == (8, C, 32, 32)

    w_qkv = np.asarray(w_qkv, np.float32)
    b_qkv = np.asarray(b_qkv, np.float32)
    w_fc = np.asarray(w_fc, np.float32)
    b_fc = np.asarray(b_fc, np.float32)
    g1 = np.asarray(g1, np.float32)
    beta1 = np.asarray(beta1, np.float32)
    g2 = np.asarray(g2, np.float32)
    beta2 = np.asarray(beta2, np.float32)
    wq = g1[:, None] * w_qkv
    bq = beta1 @ w_qkv + b_qkv
    # 16-interleave v out-channels (head = p//16 within every 128-chunk) and
    # permute proj rows to match
    old_of_new = np.array([96 * ((n % 128) // 16) + 16 * (n // 128)
                           + (n % 16) for n in range(C)])
    wq[:, 1536:] = wq[:, 1536 + old_of_new]
    bq[1536:] = bq[1536 + old_of_new]
    w_proj = np.asarray(w_proj, np.float32)[old_of_new, :]
    wf = g2[:, None] * w_fc
    bfc2 = beta2 @ w_fc + b_fc

    sels = _build_sels()
    shared = dict(
        wqkv=np.ascontiguousarray(wq.reshape(NCH, 128, 2304)).astype(bf),
        wproj=np.ascontiguousarray(
            np.asarray(w_proj, np.float32).reshape(NCH, 128, 768)).astype(bf),
        wfc=np.ascontiguousarray(wf.reshape(NCH, 128, 256)).astype(bf),
        bqkv=np.ascontiguousarray(bq.reshape(18, 128).T),
        bproj=np.ascontiguousarray(
            np.asarray(b_proj, np.float32).reshape(NCH, 128).T),
        bfc=np.ascontiguousarray(bfc2.reshape(2, 128).T),
        **sels,
    )
    in_maps = []
    for b in range(B):
        xpad = np.pad(x[b], ((0, 0), (1, 1), (1, 1)), mode="edge")
        xp = np.ascontiguousarray(xpad.reshape(NCH, 128, A)).astype(bf)
        in_maps.append(dict(xp=xp, **shared))

    nc = _build_module()
    res = run_bass_kernel_spmd(nc, in_maps, core_ids=list(range(8)),
                               **(_run_kwargs or {}))
    outs = []
    for b in range(B):
        y = np.asarray(res.results[b]["y"], np.float32)  # [2,128,32,32]
        outs.append(y.reshape(256, 32, 32))
    out = np.stack(outs).astype(np.float32)
    if _run_kwargs is not None:
        kernel.last_result = res
    return out
